# revision 8
# baseline (speedup 1.0000x reference)
"""Paged-attention decode kernel for Trainium2 (Bass/Tile), 8 NeuronCores.

Sharding: one KV head per core (N_KV=8). Each core gets x^T plus its head's
slices of the weights and of the paged K/V caches, computes its 4 query
heads' attention and a partial output projection [B, D]; the host sums the
partials.

v3 (vs the f32r baseline):
- everything bf16 on the wire (weights, x, K, V, probabilities): halves HBM
  traffic and removes the f32r 2x LDWEIGHTS / 4x small-moving-matmul
  penalties.
- projections flipped: x^T chunk is the (32-col, cheap) stationary operand,
  fused [Wq|Wk|Wv] is the moving operand -> 64 matmuls instead of 384, and
  no 128-col LDWEIGHTS for weight tiles.
- the new token's k/v are spliced into the gathered K^T/V SBUF tiles (one
  DVE column copy + one SWDGE row copy per request) instead of 4 extra
  matmuls per request.
- softmax denominators for all 32 requests are computed by 17 batched
  matmuls over one shared probability tile (ones-vector contraction) and
  applied as a single [128,128] normalize at the end, instead of per-request
  per-chunk denominator matmuls + 32 reciprocal/mul pairs.
"""
import os
import sys
from contextlib import ExitStack

import numpy as np

for _p in ("/opt/trn_rl_repo", "/opt/pypackages"):
    if os.path.isdir(_p) and _p not in sys.path:
        sys.path.append(_p)

import concourse.bass as bass  # noqa: E402,F401
import concourse.tile as tile  # noqa: E402
from concourse import bacc, mybir  # noqa: E402
from concourse.bass_utils import run_bass_kernel_spmd  # noqa: E402

N_HEADS = 32
N_KV = 8
HEAD_DIM = 128
BLOCK_SIZE = 16
MAX_SEQ = 2048
ROPE_BASE = 10000.0
SCALE = HEAD_DIM ** -0.5
B = 32
D = 4096
G = N_HEADS // N_KV  # 4 query heads per kv head
GD = G * HEAD_DIM    # 512
N_CORES = 8
NGRP = B * MAX_SEQ // 128  # 512 slot groups
MAX_CH = MAX_SEQ // 128    # 16
NSUB = MAX_CH + 1          # score sub-blocks incl. tail slack: 17
WQKV = GD + 2 * HEAD_DIM   # 768 fused projection cols per input chunk

F32 = mybir.dt.float32
BF16 = mybir.dt.bfloat16

LAST_RESULTS = None  # test harness reads exec_time_ns from here


def _kv_blocks(bt_row, L):
    """16-slot gather blocks [(slot_start, t_start, n_rows)] covering
    t in [0, L), runs coalesced."""
    nblocks = (L + BLOCK_SIZE - 1) // BLOCK_SIZE
    runs = []
    cur_s = cur_t = cur_n = 0
    for j in range(nblocks):
        rows = min(BLOCK_SIZE, L - j * BLOCK_SIZE)
        s = int(bt_row[j]) * BLOCK_SIZE
        if cur_n and s == cur_s + cur_n:
            cur_n += rows
        else:
            if cur_n:
                runs.append((cur_s, cur_t, cur_n))
            cur_s, cur_t, cur_n = s, j * BLOCK_SIZE, rows
    if cur_n:
        runs.append((cur_s, cur_t, cur_n))
    return runs


def _emit_kv_dmas(engs, kt_d, vv_d, KTt, Vt, runs, ei=0):
    """Gather the relaid caches: K^T (partition=d) and V (partition=slot%128)
    for chunk c land at cols [c*128, (c+1)*128)."""
    def dma(dst, srca):
        nonlocal ei
        engs[ei % len(engs)].dma_start(dst, srca)
        ei += 1

    for (s0, t0, n) in runs:
        if (s0 % 128 == 0 and t0 % 128 == 0
                and (s0 + n + 127) // 128 <= NGRP):
            # contiguous aligned run: K as one flat DMA; V rounded up to
            # whole 128-slot groups (over-read stays in-bounds, and any
            # garbage tail rows sit beyond Lv so they are never read)
            nf = (n + 127) // 128
            g0, c0 = s0 // 128, t0 // 128
            dma(KTt[:, t0:t0 + n], kt_d[:, s0:s0 + n])
            dma(Vt[:, c0 * 128:(c0 + nf) * 128],
                vv_d[:, g0 * 128:(g0 + nf) * 128])
            continue
        while n > 0:
            if s0 % 128 == 0 and t0 % 128 == 0 and n >= 128:
                nfull = n // 128
                g0, c0 = s0 // 128, t0 // 128
                dma(KTt[:, c0 * 128:(c0 + nfull) * 128],
                    kt_d[:, g0 * 128:(g0 + nfull) * 128])
                dma(Vt[:, c0 * 128:(c0 + nfull) * 128],
                    vv_d[:, g0 * 128:(g0 + nfull) * 128])
                take = nfull * 128
            else:
                g0, o = s0 // 128, s0 % 128
                c0, to = t0 // 128, t0 % 128
                take = min(n, 128 - o, 128 - to)
                dma(KTt[:, c0 * 128 + to:c0 * 128 + to + take],
                    kt_d[:, g0 * 128 + o:g0 * 128 + o + take])
                dma(Vt[to:to + take, c0 * 128:(c0 + 1) * 128],
                    vv_d[o:o + take, g0 * 128:(g0 + 1) * 128])
            s0 += take
            t0 += take
            n -= take
    return ei


def _build_nc(Ls, runs_all):
    nc = bacc.Bacc("TRN2", target_bir_lowering=False, debug=False,
                   num_devices=N_CORES)

    xt_d = nc.declare_dram_parameter("xT", [128, 32 * B], BF16, isOutput=False)
    wqkv_d = nc.declare_dram_parameter("wqkv", [128, 32 * WQKV], BF16,
                                       isOutput=False)
    wo_d = nc.declare_dram_parameter("wo", [GD, D], BF16, isOutput=False)
    kt_d = nc.declare_dram_parameter("kt", [128, NGRP * 128], BF16,
                                     isOutput=False)
    vv_d = nc.declare_dram_parameter("vv", [128, NGRP * 128], BF16,
                                     isOutput=False)
    cqb_d = nc.declare_dram_parameter("cqb", [B, 64], F32, isOutput=False)
    sqb_d = nc.declare_dram_parameter("sqb", [B, 64], F32, isOutput=False)
    idb_d = nc.declare_dram_parameter("identb", [128, 128], BF16,
                                      isOutput=False)
    out_d = nc.declare_dram_parameter("out", [B, D], F32, isOutput=True)

    with tile.TileContext(nc) as tc, ExitStack() as top:
        cpool = top.enter_context(tc.tile_pool(name="const", bufs=1))
        qT = cpool.tile([128, G * B], BF16, tag="qT")    # [d, g*32+b] roped
        knT = cpool.tile([128, B], BF16, tag="knT")      # [d, b] roped new k
        vn = cpool.tile([B, HEAD_DIM], BF16, tag="vn")   # [b, d] new v
        pvn = cpool.tile([128, 128], BF16, tag="pvn")    # raw pv [b*4+g, d]
        pvT = cpool.tile([128, 128], BF16, tag="pvT")    # [d, b*4+g]
        sc_all = cpool.tile([128, B * NSUB * G], BF16, tag="SC")
        onesD = cpool.tile([128, 1], BF16, tag="ones")
        nc.vector.memset(onesD[:], 1.0)
        idb = cpool.tile([128, 128], BF16, tag="identb")
        nc.sync.dma_start(idb[:], idb_d[:])

        kvpool = top.enter_context(tc.tile_pool(name="KV", bufs=6))
        nrmpool = top.enter_context(tc.tile_pool(name="nrm", bufs=4))
        wop = top.enter_context(tc.tile_pool(name="wo", bufs=4))
        state = {}
        wo_tiles = []
        dma_rr = [0]

        def emit_load(b):
            KTt = kvpool.tile([128, MAX_CH * 128], BF16, tag="KT",
                              name=f"KTt{b}")
            Vt = kvpool.tile([128, MAX_CH * 128], BF16, tag="V",
                             name=f"Vt{b}")
            dma_rr[0] = _emit_kv_dmas([nc.sync], kt_d, vv_d,
                                      KTt, Vt, runs_all[b], dma_rr[0])
            state[b] = (KTt, Vt)

        def emit_wo_load(g):
            wo_t = wop.tile([128, D], BF16, tag="wo", name=f"wo{g}")
            nc.gpsimd.dma_start(wo_t[:], wo_d[g * 128:(g + 1) * 128, :])
            wo_tiles.append(wo_t)

        # ---- phase 1: fused q/k/v projection (x^T stationary) + rope -----
        with ExitStack() as s1:
            p1 = s1.enter_context(tc.tile_pool(name="p1", bufs=1))
            wqp = s1.enter_context(tc.tile_pool(name="wqp", bufs=4))
            ps_q = s1.enter_context(
                tc.tile_pool(name="ps_q", bufs=1, space="PSUM"))
            ps_kv = s1.enter_context(
                tc.tile_pool(name="ps_kv", bufs=1, space="PSUM"))
            ps_tr = s1.enter_context(
                tc.tile_pool(name="ps_tr", bufs=2, space="PSUM"))
            tmp = s1.enter_context(tc.tile_pool(name="rtmp", bufs=4))

            xT = p1.tile([128, 32 * B], BF16, tag="xT")   # [d, kc*32+b]
            nc.sync.dma_start(xT[:], xt_d[:])
            cqb = p1.tile([B, 64], F32, tag="cqb")
            sqb = p1.tile([B, 64], F32, tag="sqb")
            nc.scalar.dma_start(cqb[:], cqb_d[:])
            nc.scalar.dma_start(sqb[:], sqb_d[:])

            q_ps = ps_q.tile([B, GD], F32, tag="ps_q")
            kv_ps = ps_kv.tile([B, 2 * HEAD_DIM], F32, tag="ps_kv")
            for j in range(4):
                wqkv_t = wqp.tile([128, 8 * WQKV], BF16, tag="wqkv",
                                  name=f"wqkv{j}")
                eng = nc.sync if j % 2 == 0 else nc.scalar
                eng.dma_start(wqkv_t[:],
                              wqkv_d[:, j * 8 * WQKV:(j + 1) * 8 * WQKV])
                for ci in range(8):
                    kc = j * 8 + ci
                    lx = xT[:, kc * B:(kc + 1) * B]
                    nc.tensor.matmul(q_ps[:],
                                     lx, wqkv_t[:, ci * WQKV:ci * WQKV + GD],
                                     start=(kc == 0), stop=(kc == 31))
                    nc.tensor.matmul(
                        kv_ps[:], lx,
                        wqkv_t[:, ci * WQKV + GD:(ci + 1) * WQKV],
                        start=(kc == 0), stop=(kc == 31))

            emit_load(0)
            emit_load(1)
            emit_load(2)

            # rope in [b, d] layout: halves d<64 / d>=64, then transpose
            qro = p1.tile([B, GD], BF16, tag="qro")
            kro = p1.tile([B, HEAD_DIM], BF16, tag="kro")

            def rope_b(src, o0, o1):
                t1 = tmp.tile([B, 64], F32, tag="rt1", name="t1")
                t2 = tmp.tile([B, 64], F32, tag="rt2", name="t2")
                nc.vector.tensor_mul(t1[:], src[:, 0:64], cqb[:])
                nc.vector.tensor_mul(t2[:], src[:, 64:128], sqb[:])
                nc.vector.tensor_sub(o0, t1[:], t2[:])
                t3 = tmp.tile([B, 64], F32, tag="rt1", name="t3")
                t4 = tmp.tile([B, 64], F32, tag="rt2", name="t4")
                nc.vector.tensor_mul(t3[:], src[:, 0:64], sqb[:])
                nc.vector.tensor_mul(t4[:], src[:, 64:128], cqb[:])
                nc.vector.tensor_add(o1, t3[:], t4[:])

            for g in range(G):
                rope_b(q_ps[:, g * 128:(g + 1) * 128],
                       qro[:, g * 128:g * 128 + 64],
                       qro[:, g * 128 + 64:(g + 1) * 128])
            rope_b(kv_ps[:, 0:128], kro[:, 0:64], kro[:, 64:128])
            nc.vector.tensor_copy(vn[:], kv_ps[:, 128:256])

            for g in range(G):
                trq = ps_tr.tile([128, B], BF16, tag="ps_tr", name=f"trq{g}")
                nc.tensor.transpose(trq[:], qro[:, g * 128:(g + 1) * 128],
                                    idb[0:B, 0:B])
                nc.vector.tensor_copy(qT[:, g * B:(g + 1) * B], trq[:])
            trk = ps_tr.tile([128, B], BF16, tag="ps_tr", name="trk")
            nc.tensor.transpose(trk[:], kro[:], idb[0:B, 0:B])
            nc.vector.tensor_copy(knT[:], trk[:])

        # ---- phase 2: per-request attention ------------------------------
        with ExitStack() as s3:
            ps_qk = s3.enter_context(
                tc.tile_pool(name="ps_qk", bufs=3, space="PSUM"))
            ps_pv = s3.enter_context(
                tc.tile_pool(name="ps_pv", bufs=2, space="PSUM"))

            def emit_qk_chunk(b, qk, rq, c):
                L = Ls[b]
                KTt, _ = state[b]
                Lvc = min(128, L - c * 128)
                nc.tensor.matmul(qk[0:Lvc, c * G:(c + 1) * G],
                                 KTt[:, c * 128:c * 128 + Lvc], rq,
                                 start=True, stop=True)

            def emit_pv_chunk(b, pv, c, nch2):
                L = Ls[b]
                _, Vt = state[b]
                Lvc = min(128, L - c * 128)
                nc.tensor.matmul(pv[:],
                                 sc_all[0:Lvc, c * B * G + b * G:
                                        c * B * G + (b + 1) * G],
                                 Vt[0:Lvc, c * 128:(c + 1) * 128],
                                 start=(c == 0), stop=(c == nch2 - 1))

            def emit_qk_prologue(b):
                L = Ls[b]
                nch2 = (L + 127) // 128
                Lv = L - (nch2 - 1) * 128
                KTt, Vt = state[b]
                Lg = L - 1
                # splice the new token's k/v into the gathered tiles
                nc.vector.tensor_copy(KTt[:, Lg:Lg + 1], knT[:, b:b + 1])
                nc.gpsimd.dma_start(
                    Vt[Lg % 128:Lg % 128 + 1,
                       (Lg // 128) * 128:(Lg // 128 + 1) * 128],
                    vn[b:b + 1, :])
                qk = ps_qk.tile([128, NSUB * G], F32, tag="ps_qk",
                                name=f"qk{b}")
                rq = qT[:].rearrange("p (g b) -> p g b", b=B)[:, :, b]
                if Lv < 128:
                    # pre-fill the tail chunk's block; the matmul then
                    # overwrites rows [0:Lv), leaving -inf beyond the
                    # context (PSUM accesses must be 32-partition-aligned,
                    # so a [Lv:128) memset is not expressible)
                    nc.vector.memset(qk[:, (nch2 - 1) * G:nch2 * G], -1e30)
                return qk, rq, nch2

            def emit_exp(b, qk, nch2):
                scv = sc_all[:].rearrange("p (c w) -> p c w", w=B * G)
                if nch2 < NSUB:
                    nc.vector.memset(
                        scv[:, nch2:NSUB, b * G:(b + 1) * G], 0.0)
                nc.scalar.activation(scv[:, 0:nch2, b * G:(b + 1) * G],
                                     qk[:, 0:nch2 * G],
                                     mybir.ActivationFunctionType.Exp,
                                     scale=SCALE)

            def emit_pv_epilogue(b, pv):
                state.pop(b)
                att = nrmpool.tile([G, HEAD_DIM], BF16, tag="att",
                                   name=f"att{b}")
                nc.vector.tensor_copy(att[:], pv[:])
                nc.gpsimd.dma_start(pvn[G * b:G * (b + 1), :], att[:])

            # software pipeline: QK(b) chunk matmuls interleave with
            # PV(b-1) chunk matmuls so QK's 128-col LDWEIGHTS hides under
            # PV's 128-col moving stream (and vice versa)
            prev = None  # (b, pv_tile, nch)
            for b in range(B):
                if b + 3 < B:
                    emit_load(b + 3)
                if b in (6, 11, 16, 21):
                    emit_wo_load((b - 6) // 5)
                qk, rq, nch2 = emit_qk_prologue(b)
                if prev is None:
                    for c in range(nch2):
                        emit_qk_chunk(b, qk, rq, c)
                else:
                    pb, pv, pnch = prev
                    for c in range(max(nch2, pnch)):
                        if c < nch2:
                            emit_qk_chunk(b, qk, rq, c)
                        if c < pnch:
                            emit_pv_chunk(pb, pv, c, pnch)
                    emit_pv_epilogue(pb, pv)
                emit_exp(b, qk, nch2)
                pv = ps_pv.tile([G, HEAD_DIM], F32, tag="ps_pv",
                                name=f"pv{b}")
                prev = (b, pv, nch2)
            pb, pv, pnch = prev
            for c in range(pnch):
                emit_pv_chunk(pb, pv, c, pnch)
            emit_pv_epilogue(pb, pv)

        # ---- phase 3: batched softmax denominators + normalize -----------
        with ExitStack() as s4:
            npool = s4.enter_context(tc.tile_pool(name="norm", bufs=1))
            ps_den = s4.enter_context(
                tc.tile_pool(name="ps_den", bufs=1, space="PSUM"))
            ps_t = s4.enter_context(
                tc.tile_pool(name="ps_t", bufs=1, space="PSUM"))
            den_ps = ps_den.tile([128, 1], F32, tag="ps_den")
            for c in range(NSUB):
                nc.tensor.matmul(den_ps[:],
                                 sc_all[:, c * B * G:(c + 1) * B * G],
                                 onesD[:],
                                 start=(c == 0), stop=(c == NSUB - 1))
            rcp = npool.tile([128, 1], F32, tag="rcp")
            nc.vector.reciprocal(rcp[:], den_ps[:])
            pvm = npool.tile([128, 128], BF16, tag="pvm")
            nc.vector.tensor_scalar_mul(pvm[:], pvn[:], rcp[:])
            pvt_ps = ps_t.tile([128, 128], BF16, tag="ps_t")
            nc.tensor.transpose(pvt_ps[:], pvm[:], idb[:])
            nc.vector.tensor_copy(pvT[:], pvt_ps[:])

        # ---- phase 4: output projection ----------------------------------
        with ExitStack() as s5:
            outp = s5.enter_context(tc.tile_pool(name="outp", bufs=1))
            ps_o = s5.enter_context(
                tc.tile_pool(name="ps_o", bufs=8, space="PSUM"))
            out_sb = outp.tile([B, D], F32, tag="out")
            o_ps = [ps_o.tile([B, 512], F32, tag="ps_o", name=f"ops{n}")
                    for n in range(8)]
            pvr = pvT[:].rearrange("p (b g) -> p b g", g=G)
            for g in range(G):
                lt = pvr[:, :, g]
                for n in range(8):
                    nc.tensor.matmul(o_ps[n][:], lt,
                                     wo_tiles[g][:, n * 512:(n + 1) * 512],
                                     start=(g == 0), stop=(g == G - 1))
            for n in range(8):
                if n % 2 == 0:
                    nc.vector.tensor_copy(out_sb[:, n * 512:(n + 1) * 512],
                                          o_ps[n][:])
                else:
                    nc.scalar.copy(out_sb[:, n * 512:(n + 1) * 512],
                                   o_ps[n][:])
            nc.sync.dma_start(out_d[:], out_sb[:])

    nc.compile()
    return nc


def _prep_inputs(x, Wq, Wk, Wv, Wo, key_cache, value_cache, block_tables,
                 context_lens):
    import ml_dtypes
    bf16 = ml_dtypes.bfloat16
    x = np.asarray(x, dtype=np.float32).reshape(B, D)
    # xT[p, kc*32+b] = x[b, kc*128+p]
    xT = np.ascontiguousarray(
        x.reshape(B, 32, 128).transpose(2, 1, 0).reshape(128, 32 * B)
    ).astype(bf16)
    Wq = np.asarray(Wq, dtype=np.float32)
    Wk = np.asarray(Wk, dtype=np.float32)
    Wv = np.asarray(Wv, dtype=np.float32)
    Wo = np.asarray(Wo, dtype=np.float32)
    key_cache = np.asarray(key_cache, dtype=np.float32)
    value_cache = np.asarray(value_cache, dtype=np.float32)
    bt = np.asarray(block_tables, dtype=np.int64)
    cl = np.asarray(context_lens, dtype=np.int64)

    Ls = [int(v) for v in cl]
    pos = np.array([v - 1 for v in Ls], dtype=np.int64)

    # rope tables at the new token's position ([b, half] layout)
    half = HEAD_DIM // 2
    inv_freq = (1.0 / (ROPE_BASE ** (np.arange(half, dtype=np.float32) / half))
                ).astype(np.float32)
    ang = pos.astype(np.float32)[:, None] * inv_freq[None, :]
    cqb = np.ascontiguousarray(np.cos(ang).astype(np.float32))  # [B, 64]
    sqb = np.ascontiguousarray(np.sin(ang).astype(np.float32))
    identb = np.eye(128, dtype=np.float32).astype(bf16)

    # gather runs over t in [0, L-1) - the new token is spliced on device
    runs = [_kv_blocks(bt[b], Ls[b] - 1) for b in range(B)]

    in_maps = []
    for h in range(N_CORES):
        # relaid cache: row p of group g = K^T col d=p / V row t=p
        K = key_cache[:, h, :].reshape(NGRP, 128, HEAD_DIM)
        V = value_cache[:, h, :].reshape(NGRP, 128, HEAD_DIM)
        kt = np.ascontiguousarray(
            K.transpose(2, 0, 1).reshape(128, NGRP * 128)).astype(bf16)
        vv = np.ascontiguousarray(
            V.transpose(1, 0, 2).reshape(128, NGRP * 128)).astype(bf16)
        # fused [Wq | Wk | Wv] moving operand, per 128-row input chunk
        wqkv = np.empty((128, 32 * WQKV), dtype=np.float32)
        for kc in range(32):
            r0 = kc * 128
            wqkv[:, kc * WQKV:kc * WQKV + GD] = \
                Wq[r0:r0 + 128, h * GD:(h + 1) * GD]
            wqkv[:, kc * WQKV + GD:kc * WQKV + GD + HEAD_DIM] = \
                Wk[r0:r0 + 128, h * HEAD_DIM:(h + 1) * HEAD_DIM]
            wqkv[:, kc * WQKV + GD + HEAD_DIM:(kc + 1) * WQKV] = \
                Wv[r0:r0 + 128, h * HEAD_DIM:(h + 1) * HEAD_DIM]
        in_maps.append({
            "xT": xT,
            "wqkv": wqkv.astype(bf16),
            "wo": np.ascontiguousarray(Wo[h * GD:(h + 1) * GD, :]
                                       ).astype(bf16),
            "kt": kt, "vv": vv,
            "cqb": cqb, "sqb": sqb, "identb": identb,
        })
    return Ls, runs, in_maps


def kernel(x, Wq, Wk, Wv, Wo, key_cache, value_cache, block_tables,
           context_lens):
    global LAST_RESULTS
    Ls, runs, in_maps = _prep_inputs(
        x, Wq, Wk, Wv, Wo, key_cache, value_cache, block_tables,
        context_lens)
    nc = _build_nc(Ls, runs)
    res = run_bass_kernel_spmd(nc, in_maps, list(range(N_CORES)))
    LAST_RESULTS = res

    out = np.zeros((B, D), dtype=np.float32)
    for h in range(N_CORES):
        out += res.results[h]["out"]
    return np.ascontiguousarray(out.reshape(B, 1, D))


# revision 9
# speedup vs baseline: 1.0725x; 1.0725x over previous
"""Paged-attention decode kernel for Trainium2 (Bass/Tile), 8 NeuronCores.

Sharding: one KV head per core (N_KV=8). Each core gets x^T plus its head's
slices of the weights and of the paged K/V caches, computes its 4 query
heads' attention and a partial output projection [B, D]; the host sums the
partials.

v3 (vs the f32r baseline):
- everything bf16 on the wire (weights, x, K, V, probabilities): halves HBM
  traffic and removes the f32r 2x LDWEIGHTS / 4x small-moving-matmul
  penalties.
- projections flipped: x^T chunk is the (32-col, cheap) stationary operand,
  fused [Wq|Wk|Wv] is the moving operand -> 64 matmuls instead of 384, and
  no 128-col LDWEIGHTS for weight tiles.
- the new token's k/v are spliced into the gathered K^T/V SBUF tiles (one
  DVE column copy + one SWDGE row copy per request) instead of 4 extra
  matmuls per request.
- softmax denominators for all 32 requests are computed by 17 batched
  matmuls over one shared probability tile (ones-vector contraction) and
  applied as a single [128,128] normalize at the end, instead of per-request
  per-chunk denominator matmuls + 32 reciprocal/mul pairs.
"""
import os
import sys
from contextlib import ExitStack

import numpy as np

for _p in ("/opt/trn_rl_repo", "/opt/pypackages"):
    if os.path.isdir(_p) and _p not in sys.path:
        sys.path.append(_p)

import concourse.bass as bass  # noqa: E402,F401
import concourse.tile as tile  # noqa: E402
from concourse import bacc, mybir  # noqa: E402
from concourse.bass_utils import run_bass_kernel_spmd  # noqa: E402

N_HEADS = 32
N_KV = 8
HEAD_DIM = 128
BLOCK_SIZE = 16
MAX_SEQ = 2048
ROPE_BASE = 10000.0
SCALE = HEAD_DIM ** -0.5
B = 32
D = 4096
G = N_HEADS // N_KV  # 4 query heads per kv head
GD = G * HEAD_DIM    # 512
N_CORES = 8
NGRP = B * MAX_SEQ // 128  # 512 slot groups
MAX_CH = MAX_SEQ // 128    # 16
NSUB = MAX_CH + 1          # score sub-blocks incl. tail slack: 17
WQKV = GD + 2 * HEAD_DIM   # 768 fused projection cols per input chunk

F32 = mybir.dt.float32
BF16 = mybir.dt.bfloat16

LAST_RESULTS = None  # test harness reads exec_time_ns from here


def _kv_blocks(bt_row, L):
    """16-slot gather blocks [(slot_start, t_start, n_rows)] covering
    t in [0, L), runs coalesced."""
    nblocks = (L + BLOCK_SIZE - 1) // BLOCK_SIZE
    runs = []
    cur_s = cur_t = cur_n = 0
    for j in range(nblocks):
        rows = min(BLOCK_SIZE, L - j * BLOCK_SIZE)
        s = int(bt_row[j]) * BLOCK_SIZE
        if cur_n and s == cur_s + cur_n:
            cur_n += rows
        else:
            if cur_n:
                runs.append((cur_s, cur_t, cur_n))
            cur_s, cur_t, cur_n = s, j * BLOCK_SIZE, rows
    if cur_n:
        runs.append((cur_s, cur_t, cur_n))
    return runs


def _emit_kv_dmas(engs, kt_d, vv_d, KTt, Vt, runs, ei=0):
    """Gather the relaid caches: K^T (partition=d) and V (partition=slot%128)
    for chunk c land at cols [c*128, (c+1)*128)."""
    def dma(dst, srca):
        nonlocal ei
        engs[ei % len(engs)].dma_start(dst, srca)
        ei += 1

    for (s0, t0, n) in runs:
        if (s0 % 128 == 0 and t0 % 128 == 0
                and (s0 + n + 127) // 128 <= NGRP):
            # contiguous aligned run: K as one flat DMA; V rounded up to
            # whole 128-slot groups (over-read stays in-bounds, and any
            # garbage tail rows sit beyond Lv so they are never read)
            nf = (n + 127) // 128
            g0, c0 = s0 // 128, t0 // 128
            dma(KTt[:, t0:t0 + n], kt_d[:, s0:s0 + n])
            dma(Vt[:, c0 * 128:(c0 + nf) * 128],
                vv_d[:, g0 * 128:(g0 + nf) * 128])
            continue
        while n > 0:
            if s0 % 128 == 0 and t0 % 128 == 0 and n >= 128:
                nfull = n // 128
                g0, c0 = s0 // 128, t0 // 128
                dma(KTt[:, c0 * 128:(c0 + nfull) * 128],
                    kt_d[:, g0 * 128:(g0 + nfull) * 128])
                dma(Vt[:, c0 * 128:(c0 + nfull) * 128],
                    vv_d[:, g0 * 128:(g0 + nfull) * 128])
                take = nfull * 128
            else:
                g0, o = s0 // 128, s0 % 128
                c0, to = t0 // 128, t0 % 128
                take = min(n, 128 - o, 128 - to)
                dma(KTt[:, c0 * 128 + to:c0 * 128 + to + take],
                    kt_d[:, g0 * 128 + o:g0 * 128 + o + take])
                dma(Vt[to:to + take, c0 * 128:(c0 + 1) * 128],
                    vv_d[o:o + take, g0 * 128:(g0 + 1) * 128])
            s0 += take
            t0 += take
            n -= take
    return ei


def _build_nc(Ls, runs_all):
    nc = bacc.Bacc("TRN2", target_bir_lowering=False, debug=False,
                   num_devices=N_CORES)

    xt_d = nc.declare_dram_parameter("xT", [128, 32 * B], BF16, isOutput=False)
    wqkv_d = nc.declare_dram_parameter("wqkv", [128, 32 * WQKV], BF16,
                                       isOutput=False)
    wo_d = nc.declare_dram_parameter("wo", [GD, D], BF16, isOutput=False)
    kt_d = nc.declare_dram_parameter("kt", [128, NGRP * 128], BF16,
                                     isOutput=False)
    vv_d = nc.declare_dram_parameter("vv", [128, NGRP * 128], BF16,
                                     isOutput=False)
    cqb_d = nc.declare_dram_parameter("cqb", [B, 64], F32, isOutput=False)
    sqb_d = nc.declare_dram_parameter("sqb", [B, 64], F32, isOutput=False)
    idb_d = nc.declare_dram_parameter("identb", [128, 128], BF16,
                                      isOutput=False)
    out_d = nc.declare_dram_parameter("out", [B, D], F32, isOutput=True)

    with tile.TileContext(nc) as tc, ExitStack() as top:
        cpool = top.enter_context(tc.tile_pool(name="const", bufs=1))
        qT = cpool.tile([128, G * B], BF16, tag="qT")    # [d, g*32+b] roped
        knT = cpool.tile([128, B], BF16, tag="knT")      # [d, b] roped new k
        vn = cpool.tile([B, HEAD_DIM], BF16, tag="vn")   # [b, d] new v
        pvn = cpool.tile([128, 128], BF16, tag="pvn")    # raw pv [b*4+g, d]
        pvT = cpool.tile([128, 128], BF16, tag="pvT")    # [d, b*4+g]
        sc_all = cpool.tile([128, B * NSUB * G], BF16, tag="SC")
        onesD = cpool.tile([128, 1], BF16, tag="ones")
        nc.vector.memset(onesD[:], 1.0)
        idb = cpool.tile([128, 128], BF16, tag="identb")
        nc.sync.dma_start(idb[:], idb_d[:])

        kvpool = top.enter_context(tc.tile_pool(name="KV", bufs=6))
        nrmpool = top.enter_context(tc.tile_pool(name="nrm", bufs=4))
        wop = top.enter_context(tc.tile_pool(name="wo", bufs=4))
        state = {}
        wo_tiles = []
        dma_rr = [0]

        def emit_splice(b):
            # splice the new token's k/v into the gathered tiles, well
            # ahead of the QK/PV matmuls that read them
            KTt, Vt = state[b]
            Lg = Ls[b] - 1
            nc.vector.tensor_copy(KTt[:, Lg:Lg + 1], knT[:, b:b + 1])
            nc.gpsimd.dma_start(
                Vt[Lg % 128:Lg % 128 + 1,
                   (Lg // 128) * 128:(Lg // 128 + 1) * 128],
                vn[b:b + 1, :])

        def emit_load(b, splice=True):
            KTt = kvpool.tile([128, MAX_CH * 128], BF16, tag="KT",
                              name=f"KTt{b}")
            Vt = kvpool.tile([128, MAX_CH * 128], BF16, tag="V",
                             name=f"Vt{b}")
            dma_rr[0] = _emit_kv_dmas([nc.sync, nc.scalar], kt_d, vv_d,
                                      KTt, Vt, runs_all[b], dma_rr[0])
            state[b] = (KTt, Vt)
            if splice:
                emit_splice(b)

        def emit_wo_load(g):
            wo_t = wop.tile([128, D], BF16, tag="wo", name=f"wo{g}")
            eng = nc.sync if g % 2 == 0 else nc.scalar
            eng.dma_start(wo_t[:], wo_d[g * 128:(g + 1) * 128, :])
            wo_tiles.append(wo_t)

        # ---- phase 1: fused q/k/v projection (x^T stationary) + rope -----
        with ExitStack() as s1:
            p1 = s1.enter_context(tc.tile_pool(name="p1", bufs=1))
            wqp = s1.enter_context(tc.tile_pool(name="wqp", bufs=4))
            ps_q = s1.enter_context(
                tc.tile_pool(name="ps_q", bufs=1, space="PSUM"))
            ps_kv = s1.enter_context(
                tc.tile_pool(name="ps_kv", bufs=1, space="PSUM"))
            ps_tr = s1.enter_context(
                tc.tile_pool(name="ps_tr", bufs=2, space="PSUM"))
            tmp = s1.enter_context(tc.tile_pool(name="rtmp", bufs=4))

            xT = p1.tile([128, 32 * B], BF16, tag="xT")   # [d, kc*32+b]
            nc.sync.dma_start(xT[:], xt_d[:])
            cqb = p1.tile([B, 64], F32, tag="cqb")
            sqb = p1.tile([B, 64], F32, tag="sqb")
            nc.scalar.dma_start(cqb[:], cqb_d[:])
            nc.scalar.dma_start(sqb[:], sqb_d[:])

            q_ps = ps_q.tile([B, GD], F32, tag="ps_q")
            kv_ps = ps_kv.tile([B, 2 * HEAD_DIM], F32, tag="ps_kv")
            for j in range(4):
                wqkv_t = wqp.tile([128, 8 * WQKV], BF16, tag="wqkv",
                                  name=f"wqkv{j}")
                eng = nc.sync if j % 2 == 0 else nc.scalar
                eng.dma_start(wqkv_t[:],
                              wqkv_d[:, j * 8 * WQKV:(j + 1) * 8 * WQKV])
                for ci in range(8):
                    kc = j * 8 + ci
                    lx = xT[:, kc * B:(kc + 1) * B]
                    nc.tensor.matmul(q_ps[:],
                                     lx, wqkv_t[:, ci * WQKV:ci * WQKV + GD],
                                     start=(kc == 0), stop=(kc == 31))
                    nc.tensor.matmul(
                        kv_ps[:], lx,
                        wqkv_t[:, ci * WQKV + GD:(ci + 1) * WQKV],
                        start=(kc == 0), stop=(kc == 31))

            emit_load(0, splice=False)
            emit_load(1, splice=False)
            emit_load(2, splice=False)

            # rope in [b, d] layout: halves d<64 / d>=64, then transpose
            qro = p1.tile([B, GD], BF16, tag="qro")
            kro = p1.tile([B, HEAD_DIM], BF16, tag="kro")

            def rope_b(src, o0, o1):
                t1 = tmp.tile([B, 64], F32, tag="rt1", name="t1")
                t2 = tmp.tile([B, 64], F32, tag="rt2", name="t2")
                nc.vector.tensor_mul(t1[:], src[:, 0:64], cqb[:])
                nc.vector.tensor_mul(t2[:], src[:, 64:128], sqb[:])
                nc.vector.tensor_sub(o0, t1[:], t2[:])
                t3 = tmp.tile([B, 64], F32, tag="rt1", name="t3")
                t4 = tmp.tile([B, 64], F32, tag="rt2", name="t4")
                nc.vector.tensor_mul(t3[:], src[:, 0:64], sqb[:])
                nc.vector.tensor_mul(t4[:], src[:, 64:128], cqb[:])
                nc.vector.tensor_add(o1, t3[:], t4[:])

            for g in range(G):
                rope_b(q_ps[:, g * 128:(g + 1) * 128],
                       qro[:, g * 128:g * 128 + 64],
                       qro[:, g * 128 + 64:(g + 1) * 128])
            rope_b(kv_ps[:, 0:128], kro[:, 0:64], kro[:, 64:128])
            nc.vector.tensor_copy(vn[:], kv_ps[:, 128:256])

            for g in range(G):
                trq = ps_tr.tile([128, B], BF16, tag="ps_tr", name=f"trq{g}")
                nc.tensor.transpose(trq[:], qro[:, g * 128:(g + 1) * 128],
                                    idb[0:B, 0:B])
                nc.vector.tensor_copy(qT[:, g * B:(g + 1) * B], trq[:])
            trk = ps_tr.tile([128, B], BF16, tag="ps_tr", name="trk")
            nc.tensor.transpose(trk[:], kro[:], idb[0:B, 0:B])
            nc.vector.tensor_copy(knT[:], trk[:])
            for b in range(3):
                emit_splice(b)

        # ---- phase 2: per-request attention ------------------------------
        with ExitStack() as s3:
            ps_qk = s3.enter_context(
                tc.tile_pool(name="ps_qk", bufs=3, space="PSUM"))
            ps_pv = s3.enter_context(
                tc.tile_pool(name="ps_pv", bufs=2, space="PSUM"))

            def emit_qk_chunk(b, qk, rq, c):
                L = Ls[b]
                KTt, _ = state[b]
                Lvc = min(128, L - c * 128)
                nc.tensor.matmul(qk[0:Lvc, c * G:(c + 1) * G],
                                 KTt[:, c * 128:c * 128 + Lvc], rq,
                                 start=True, stop=True)

            def emit_pv_chunk(b, pv, c, nch2):
                L = Ls[b]
                _, Vt = state[b]
                Lvc = min(128, L - c * 128)
                nc.tensor.matmul(pv[:],
                                 sc_all[0:Lvc, c * B * G + b * G:
                                        c * B * G + (b + 1) * G],
                                 Vt[0:Lvc, c * 128:(c + 1) * 128],
                                 start=(c == 0), stop=(c == nch2 - 1))

            def emit_qk_prologue(b):
                L = Ls[b]
                nch2 = (L + 127) // 128
                Lv = L - (nch2 - 1) * 128
                qk = ps_qk.tile([128, NSUB * G], F32, tag="ps_qk",
                                name=f"qk{b}")
                rq = qT[:].rearrange("p (g b) -> p g b", b=B)[:, :, b]
                if Lv < 128:
                    # pre-fill the tail chunk's block; the matmul then
                    # overwrites rows [0:Lv), leaving -inf beyond the
                    # context (PSUM accesses must be 32-partition-aligned,
                    # so a [Lv:128) memset is not expressible)
                    nc.vector.memset(qk[:, (nch2 - 1) * G:nch2 * G], -1e30)
                return qk, rq, nch2

            def emit_exp(b, qk, nch2):
                scv = sc_all[:].rearrange("p (c w) -> p c w", w=B * G)
                if nch2 < NSUB:
                    nc.vector.memset(
                        scv[:, nch2:NSUB, b * G:(b + 1) * G], 0.0)
                nc.scalar.activation(scv[:, 0:nch2, b * G:(b + 1) * G],
                                     qk[:, 0:nch2 * G],
                                     mybir.ActivationFunctionType.Exp,
                                     scale=SCALE)

            def emit_pv_epilogue(b, pv):
                state.pop(b)
                att = nrmpool.tile([G, HEAD_DIM], BF16, tag="att",
                                   name=f"att{b}")
                nc.vector.tensor_copy(att[:], pv[:])
                nc.gpsimd.dma_start(pvn[G * b:G * (b + 1), :], att[:])

            # software pipeline: QK(b) chunk matmuls interleave with
            # PV(b-1) chunk matmuls so QK's 128-col LDWEIGHTS hides under
            # PV's 128-col moving stream (and vice versa)
            prev = None  # (b, pv_tile, nch)
            for b in range(B):
                if b + 3 < B:
                    emit_load(b + 3)
                if b in (6, 11, 16, 21):
                    emit_wo_load((b - 6) // 5)
                qk, rq, nch2 = emit_qk_prologue(b)
                if prev is None:
                    for c in range(nch2):
                        emit_qk_chunk(b, qk, rq, c)
                else:
                    pb, pv, pnch = prev
                    for c in range(max(nch2, pnch)):
                        if c < nch2:
                            emit_qk_chunk(b, qk, rq, c)
                        if c < pnch:
                            emit_pv_chunk(pb, pv, c, pnch)
                    emit_pv_epilogue(pb, pv)
                emit_exp(b, qk, nch2)
                pv = ps_pv.tile([G, HEAD_DIM], F32, tag="ps_pv",
                                name=f"pv{b}")
                prev = (b, pv, nch2)
            pb, pv, pnch = prev
            for c in range(pnch):
                emit_pv_chunk(pb, pv, c, pnch)
            emit_pv_epilogue(pb, pv)

        # ---- phase 3: batched softmax denominators + normalize -----------
        with ExitStack() as s4:
            npool = s4.enter_context(tc.tile_pool(name="norm", bufs=1))
            ps_den = s4.enter_context(
                tc.tile_pool(name="ps_den", bufs=1, space="PSUM"))
            ps_t = s4.enter_context(
                tc.tile_pool(name="ps_t", bufs=1, space="PSUM"))
            den_ps = ps_den.tile([128, 1], F32, tag="ps_den")
            for c in range(NSUB):
                nc.tensor.matmul(den_ps[:],
                                 sc_all[:, c * B * G:(c + 1) * B * G],
                                 onesD[:],
                                 start=(c == 0), stop=(c == NSUB - 1))
            rcp = npool.tile([128, 1], F32, tag="rcp")
            nc.vector.reciprocal(rcp[:], den_ps[:])
            pvm = npool.tile([128, 128], BF16, tag="pvm")
            nc.vector.tensor_scalar_mul(pvm[:], pvn[:], rcp[:])
            pvt_ps = ps_t.tile([128, 128], BF16, tag="ps_t")
            nc.tensor.transpose(pvt_ps[:], pvm[:], idb[:])
            nc.vector.tensor_copy(pvT[:], pvt_ps[:])

        # ---- phase 4: output projection ----------------------------------
        with ExitStack() as s5:
            outp = s5.enter_context(tc.tile_pool(name="outp", bufs=1))
            ps_o = s5.enter_context(
                tc.tile_pool(name="ps_o", bufs=8, space="PSUM"))
            out_sb = outp.tile([B, D], F32, tag="out")
            o_ps = [ps_o.tile([B, 512], F32, tag="ps_o", name=f"ops{n}")
                    for n in range(8)]
            pvr = pvT[:].rearrange("p (b g) -> p b g", g=G)
            for g in range(G):
                lt = pvr[:, :, g]
                for n in range(8):
                    nc.tensor.matmul(o_ps[n][:], lt,
                                     wo_tiles[g][:, n * 512:(n + 1) * 512],
                                     start=(g == 0), stop=(g == G - 1))
            for n in range(8):
                if n % 2 == 0:
                    nc.vector.tensor_copy(out_sb[:, n * 512:(n + 1) * 512],
                                          o_ps[n][:])
                else:
                    nc.scalar.copy(out_sb[:, n * 512:(n + 1) * 512],
                                   o_ps[n][:])
            nc.sync.dma_start(out_d[:], out_sb[:])

    nc.compile()
    return nc


def _prep_inputs(x, Wq, Wk, Wv, Wo, key_cache, value_cache, block_tables,
                 context_lens):
    import ml_dtypes
    bf16 = ml_dtypes.bfloat16
    x = np.asarray(x, dtype=np.float32).reshape(B, D)
    # xT[p, kc*32+b] = x[b, kc*128+p]
    xT = np.ascontiguousarray(
        x.reshape(B, 32, 128).transpose(2, 1, 0).reshape(128, 32 * B)
    ).astype(bf16)
    Wq = np.asarray(Wq, dtype=np.float32)
    Wk = np.asarray(Wk, dtype=np.float32)
    Wv = np.asarray(Wv, dtype=np.float32)
    Wo = np.asarray(Wo, dtype=np.float32)
    key_cache = np.asarray(key_cache, dtype=np.float32)
    value_cache = np.asarray(value_cache, dtype=np.float32)
    bt = np.asarray(block_tables, dtype=np.int64)
    cl = np.asarray(context_lens, dtype=np.int64)

    Ls = [int(v) for v in cl]
    pos = np.array([v - 1 for v in Ls], dtype=np.int64)

    # rope tables at the new token's position ([b, half] layout)
    half = HEAD_DIM // 2
    inv_freq = (1.0 / (ROPE_BASE ** (np.arange(half, dtype=np.float32) / half))
                ).astype(np.float32)
    ang = pos.astype(np.float32)[:, None] * inv_freq[None, :]
    cqb = np.ascontiguousarray(np.cos(ang).astype(np.float32))  # [B, 64]
    sqb = np.ascontiguousarray(np.sin(ang).astype(np.float32))
    identb = np.eye(128, dtype=np.float32).astype(bf16)

    # gather runs over t in [0, L-1) - the new token is spliced on device
    runs = [_kv_blocks(bt[b], Ls[b] - 1) for b in range(B)]

    in_maps = []
    for h in range(N_CORES):
        # relaid cache: row p of group g = K^T col d=p / V row t=p
        K = key_cache[:, h, :].reshape(NGRP, 128, HEAD_DIM)
        V = value_cache[:, h, :].reshape(NGRP, 128, HEAD_DIM)
        kt = np.ascontiguousarray(
            K.transpose(2, 0, 1).reshape(128, NGRP * 128)).astype(bf16)
        vv = np.ascontiguousarray(
            V.transpose(1, 0, 2).reshape(128, NGRP * 128)).astype(bf16)
        # fused [Wq | Wk | Wv] moving operand, per 128-row input chunk
        wqkv = np.empty((128, 32 * WQKV), dtype=np.float32)
        for kc in range(32):
            r0 = kc * 128
            wqkv[:, kc * WQKV:kc * WQKV + GD] = \
                Wq[r0:r0 + 128, h * GD:(h + 1) * GD]
            wqkv[:, kc * WQKV + GD:kc * WQKV + GD + HEAD_DIM] = \
                Wk[r0:r0 + 128, h * HEAD_DIM:(h + 1) * HEAD_DIM]
            wqkv[:, kc * WQKV + GD + HEAD_DIM:(kc + 1) * WQKV] = \
                Wv[r0:r0 + 128, h * HEAD_DIM:(h + 1) * HEAD_DIM]
        in_maps.append({
            "xT": xT,
            "wqkv": wqkv.astype(bf16),
            "wo": np.ascontiguousarray(Wo[h * GD:(h + 1) * GD, :]
                                       ).astype(bf16),
            "kt": kt, "vv": vv,
            "cqb": cqb, "sqb": sqb, "identb": identb,
        })
    return Ls, runs, in_maps


def kernel(x, Wq, Wk, Wv, Wo, key_cache, value_cache, block_tables,
           context_lens):
    global LAST_RESULTS
    Ls, runs, in_maps = _prep_inputs(
        x, Wq, Wk, Wv, Wo, key_cache, value_cache, block_tables,
        context_lens)
    nc = _build_nc(Ls, runs)
    res = run_bass_kernel_spmd(nc, in_maps, list(range(N_CORES)))
    LAST_RESULTS = res

    out = np.zeros((B, D), dtype=np.float32)
    for h in range(N_CORES):
        out += res.results[h]["out"]
    return np.ascontiguousarray(out.reshape(B, 1, D))


# revision 10
# speedup vs baseline: 1.0802x; 1.0072x over previous
"""Paged-attention decode kernel for Trainium2 (Bass/Tile), 8 NeuronCores.

Sharding: one KV head per core (N_KV=8). Each core gets x^T plus its head's
slices of the weights and of the paged K/V caches, computes its 4 query
heads' attention and a partial output projection [B, D]; the host sums the
partials.

v3 (vs the f32r baseline):
- everything bf16 on the wire (weights, x, K, V, probabilities): halves HBM
  traffic and removes the f32r 2x LDWEIGHTS / 4x small-moving-matmul
  penalties.
- projections flipped: x^T chunk is the (32-col, cheap) stationary operand,
  fused [Wq|Wk|Wv] is the moving operand -> 64 matmuls instead of 384, and
  no 128-col LDWEIGHTS for weight tiles.
- the new token's k/v are spliced into the gathered K^T/V SBUF tiles (one
  DVE column copy + one SWDGE row copy per request) instead of 4 extra
  matmuls per request.
- softmax denominators for all 32 requests are computed by 17 batched
  matmuls over one shared probability tile (ones-vector contraction) and
  applied as a single [128,128] normalize at the end, instead of per-request
  per-chunk denominator matmuls + 32 reciprocal/mul pairs.
"""
import os
import sys
from contextlib import ExitStack

import numpy as np

for _p in ("/opt/trn_rl_repo", "/opt/pypackages"):
    if os.path.isdir(_p) and _p not in sys.path:
        sys.path.append(_p)

import concourse.bass as bass  # noqa: E402,F401
import concourse.tile as tile  # noqa: E402
from concourse import bacc, mybir  # noqa: E402
from concourse.bass_utils import run_bass_kernel_spmd  # noqa: E402

N_HEADS = 32
N_KV = 8
HEAD_DIM = 128
BLOCK_SIZE = 16
MAX_SEQ = 2048
ROPE_BASE = 10000.0
SCALE = HEAD_DIM ** -0.5
B = 32
D = 4096
G = N_HEADS // N_KV  # 4 query heads per kv head
GD = G * HEAD_DIM    # 512
N_CORES = 8
NGRP = B * MAX_SEQ // 128  # 512 slot groups
MAX_CH = MAX_SEQ // 128    # 16
NSUB = MAX_CH + 1          # score sub-blocks incl. tail slack: 17
WQKV = GD + 2 * HEAD_DIM   # 768 fused projection cols per input chunk

F32 = mybir.dt.float32
BF16 = mybir.dt.bfloat16

LAST_RESULTS = None  # test harness reads exec_time_ns from here


def _kv_blocks(bt_row, L):
    """16-slot gather blocks [(slot_start, t_start, n_rows)] covering
    t in [0, L), runs coalesced."""
    nblocks = (L + BLOCK_SIZE - 1) // BLOCK_SIZE
    runs = []
    cur_s = cur_t = cur_n = 0
    for j in range(nblocks):
        rows = min(BLOCK_SIZE, L - j * BLOCK_SIZE)
        s = int(bt_row[j]) * BLOCK_SIZE
        if cur_n and s == cur_s + cur_n:
            cur_n += rows
        else:
            if cur_n:
                runs.append((cur_s, cur_t, cur_n))
            cur_s, cur_t, cur_n = s, j * BLOCK_SIZE, rows
    if cur_n:
        runs.append((cur_s, cur_t, cur_n))
    return runs


def _emit_kv_dmas(engs, kt_d, vv_d, KTt, Vt, runs, ei=0):
    """Gather the relaid caches: K^T (partition=d) and V (partition=slot%128)
    for chunk c land at cols [c*128, (c+1)*128)."""
    def dma(dst, srca):
        nonlocal ei
        engs[ei % len(engs)].dma_start(dst, srca)
        ei += 1

    for (s0, t0, n) in runs:
        if (s0 % 128 == 0 and t0 % 128 == 0
                and (s0 + n + 127) // 128 <= NGRP):
            # contiguous aligned run: K as one flat DMA; V rounded up to
            # whole 128-slot groups (over-read stays in-bounds, and any
            # garbage tail rows sit beyond Lv so they are never read)
            nf = (n + 127) // 128
            g0, c0 = s0 // 128, t0 // 128
            dma(KTt[:, t0:t0 + n], kt_d[:, s0:s0 + n])
            dma(Vt[:, c0 * 128:(c0 + nf) * 128],
                vv_d[:, g0 * 128:(g0 + nf) * 128])
            continue
        while n > 0:
            if s0 % 128 == 0 and t0 % 128 == 0 and n >= 128:
                nfull = n // 128
                g0, c0 = s0 // 128, t0 // 128
                dma(KTt[:, c0 * 128:(c0 + nfull) * 128],
                    kt_d[:, g0 * 128:(g0 + nfull) * 128])
                dma(Vt[:, c0 * 128:(c0 + nfull) * 128],
                    vv_d[:, g0 * 128:(g0 + nfull) * 128])
                take = nfull * 128
            else:
                g0, o = s0 // 128, s0 % 128
                c0, to = t0 // 128, t0 % 128
                take = min(n, 128 - o, 128 - to)
                dma(KTt[:, c0 * 128 + to:c0 * 128 + to + take],
                    kt_d[:, g0 * 128 + o:g0 * 128 + o + take])
                dma(Vt[to:to + take, c0 * 128:(c0 + 1) * 128],
                    vv_d[o:o + take, g0 * 128:(g0 + 1) * 128])
            s0 += take
            t0 += take
            n -= take
    return ei


def _build_nc(Ls, runs_all):
    ORDER = sorted(range(B), key=lambda b: -Ls[b])
    nc = bacc.Bacc("TRN2", target_bir_lowering=False, debug=False,
                   num_devices=N_CORES)

    xt_d = nc.declare_dram_parameter("xT", [128, 32 * B], BF16, isOutput=False)
    wqkv_d = nc.declare_dram_parameter("wqkv", [128, 32 * WQKV], BF16,
                                       isOutput=False)
    wo_d = nc.declare_dram_parameter("wo", [GD, D], BF16, isOutput=False)
    kt_d = nc.declare_dram_parameter("kt", [128, NGRP * 128], BF16,
                                     isOutput=False)
    vv_d = nc.declare_dram_parameter("vv", [128, NGRP * 128], BF16,
                                     isOutput=False)
    cqb_d = nc.declare_dram_parameter("cqb", [B, 64], F32, isOutput=False)
    sqb_d = nc.declare_dram_parameter("sqb", [B, 64], F32, isOutput=False)
    idb_d = nc.declare_dram_parameter("identb", [128, 128], BF16,
                                      isOutput=False)
    out_d = nc.declare_dram_parameter("out", [B, D], F32, isOutput=True)

    with tile.TileContext(nc) as tc, ExitStack() as top:
        cpool = top.enter_context(tc.tile_pool(name="const", bufs=1))
        qT = cpool.tile([128, G * B], BF16, tag="qT")    # [d, g*32+b] roped
        knT = cpool.tile([128, B], BF16, tag="knT")      # [d, b] roped new k
        vn = cpool.tile([B, HEAD_DIM], BF16, tag="vn")   # [b, d] new v
        pvn = cpool.tile([128, 128], BF16, tag="pvn")    # raw pv [b*4+g, d]
        pvT = cpool.tile([128, 128], BF16, tag="pvT")    # [d, b*4+g]
        sc_all = cpool.tile([128, B * NSUB * G], BF16, tag="SC")
        onesD = cpool.tile([128, 1], BF16, tag="ones")
        nc.vector.memset(onesD[:], 1.0)
        idb = cpool.tile([128, 128], BF16, tag="identb")
        nc.sync.dma_start(idb[:], idb_d[:])

        kvpool = top.enter_context(tc.tile_pool(name="KV", bufs=6))
        nrmpool = top.enter_context(tc.tile_pool(name="nrm", bufs=4))
        wop = top.enter_context(tc.tile_pool(name="wo", bufs=4))
        state = {}
        wo_tiles = []
        dma_rr = [0]

        def emit_splice(b):
            # splice the new token's k/v into the gathered tiles, well
            # ahead of the QK/PV matmuls that read them
            KTt, Vt = state[b]
            Lg = Ls[b] - 1
            nc.vector.tensor_copy(KTt[:, Lg:Lg + 1], knT[:, b:b + 1])
            nc.gpsimd.dma_start(
                Vt[Lg % 128:Lg % 128 + 1,
                   (Lg // 128) * 128:(Lg // 128 + 1) * 128],
                vn[b:b + 1, :])

        def emit_load(b, splice=True):
            KTt = kvpool.tile([128, MAX_CH * 128], BF16, tag="KT",
                              name=f"KTt{b}")
            Vt = kvpool.tile([128, MAX_CH * 128], BF16, tag="V",
                             name=f"Vt{b}")
            dma_rr[0] = _emit_kv_dmas([nc.sync, nc.scalar], kt_d, vv_d,
                                      KTt, Vt, runs_all[b], dma_rr[0])
            state[b] = (KTt, Vt)
            if splice:
                emit_splice(b)

        def emit_wo_load(g):
            wo_t = wop.tile([128, D], BF16, tag="wo", name=f"wo{g}")
            eng = nc.sync if g % 2 == 0 else nc.scalar
            eng.dma_start(wo_t[:], wo_d[g * 128:(g + 1) * 128, :])
            wo_tiles.append(wo_t)

        # ---- phase 1: fused q/k/v projection (x^T stationary) + rope -----
        with ExitStack() as s1:
            p1 = s1.enter_context(tc.tile_pool(name="p1", bufs=1))
            wqp = s1.enter_context(tc.tile_pool(name="wqp", bufs=8))
            ps_q = s1.enter_context(
                tc.tile_pool(name="ps_q", bufs=1, space="PSUM"))
            ps_kv = s1.enter_context(
                tc.tile_pool(name="ps_kv", bufs=1, space="PSUM"))
            ps_tr = s1.enter_context(
                tc.tile_pool(name="ps_tr", bufs=2, space="PSUM"))
            tmp = s1.enter_context(tc.tile_pool(name="rtmp", bufs=4))

            xT = p1.tile([128, 32 * B], BF16, tag="xT")   # [d, kc*32+b]
            nc.sync.dma_start(xT[:], xt_d[:])
            cqb = p1.tile([B, 64], F32, tag="cqb")
            sqb = p1.tile([B, 64], F32, tag="sqb")
            nc.scalar.dma_start(cqb[:], cqb_d[:])
            nc.scalar.dma_start(sqb[:], sqb_d[:])

            q_ps = ps_q.tile([B, GD], F32, tag="ps_q")
            kv_ps = ps_kv.tile([B, 2 * HEAD_DIM], F32, tag="ps_kv")
            # warm-up: keep the PE busy while the first weight chunk is in
            # flight so the clock p-state ramps before the real matmuls
            with ExitStack() as sw:
                wpool = sw.enter_context(tc.tile_pool(name="warm", bufs=1))
                ps_w = sw.enter_context(
                    tc.tile_pool(name="ps_w", bufs=1, space="PSUM"))
                wsrc = wpool.tile([128, 128], BF16, tag="wsrc")
                nc.vector.memset(wsrc[:], 0.0)
                w_ps = ps_w.tile([128, 128], F32, tag="ps_w")
                for _ in range(40):
                    nc.tensor.matmul(w_ps[:], wsrc[:], wsrc[:],
                                     start=True, stop=True)
            for j in range(8):
                wqkv_t = wqp.tile([128, 4 * WQKV], BF16, tag="wqkv",
                                  name=f"wqkv{j}")
                eng = nc.sync if j % 2 == 0 else nc.scalar
                eng.dma_start(wqkv_t[:],
                              wqkv_d[:, j * 4 * WQKV:(j + 1) * 4 * WQKV])
                for ci in range(4):
                    kc = j * 4 + ci
                    lx = xT[:, kc * B:(kc + 1) * B]
                    nc.tensor.matmul(q_ps[:],
                                     lx, wqkv_t[:, ci * WQKV:ci * WQKV + GD],
                                     start=(kc == 0), stop=(kc == 31))
                    nc.tensor.matmul(
                        kv_ps[:], lx,
                        wqkv_t[:, ci * WQKV + GD:(ci + 1) * WQKV],
                        start=(kc == 0), stop=(kc == 31))

            emit_load(ORDER[0], splice=False)
            emit_load(ORDER[1], splice=False)
            emit_load(ORDER[2], splice=False)

            # rope in [b, d] layout: halves d<64 / d>=64, then transpose
            qro = p1.tile([B, GD], BF16, tag="qro")
            kro = p1.tile([B, HEAD_DIM], BF16, tag="kro")

            def rope_b(src, o0, o1):
                t1 = tmp.tile([B, 64], F32, tag="rt1", name="t1")
                t2 = tmp.tile([B, 64], F32, tag="rt2", name="t2")
                nc.vector.tensor_mul(t1[:], src[:, 0:64], cqb[:])
                nc.vector.tensor_mul(t2[:], src[:, 64:128], sqb[:])
                nc.vector.tensor_sub(o0, t1[:], t2[:])
                t3 = tmp.tile([B, 64], F32, tag="rt1", name="t3")
                t4 = tmp.tile([B, 64], F32, tag="rt2", name="t4")
                nc.vector.tensor_mul(t3[:], src[:, 0:64], sqb[:])
                nc.vector.tensor_mul(t4[:], src[:, 64:128], cqb[:])
                nc.vector.tensor_add(o1, t3[:], t4[:])

            for g in range(G):
                rope_b(q_ps[:, g * 128:(g + 1) * 128],
                       qro[:, g * 128:g * 128 + 64],
                       qro[:, g * 128 + 64:(g + 1) * 128])
            rope_b(kv_ps[:, 0:128], kro[:, 0:64], kro[:, 64:128])
            nc.vector.tensor_copy(vn[:], kv_ps[:, 128:256])

            for g in range(G):
                trq = ps_tr.tile([128, B], BF16, tag="ps_tr", name=f"trq{g}")
                nc.tensor.transpose(trq[:], qro[:, g * 128:(g + 1) * 128],
                                    idb[0:B, 0:B])
                nc.vector.tensor_copy(qT[:, g * B:(g + 1) * B], trq[:])
            trk = ps_tr.tile([128, B], BF16, tag="ps_tr", name="trk")
            nc.tensor.transpose(trk[:], kro[:], idb[0:B, 0:B])
            nc.vector.tensor_copy(knT[:], trk[:])
            for i in range(3):
                emit_splice(ORDER[i])

        # ---- phase 2: per-request attention ------------------------------
        with ExitStack() as s3:
            ps_qk = s3.enter_context(
                tc.tile_pool(name="ps_qk", bufs=3, space="PSUM"))
            ps_pv = s3.enter_context(
                tc.tile_pool(name="ps_pv", bufs=2, space="PSUM"))

            def emit_qk_chunk(b, qk, rq, c):
                L = Ls[b]
                KTt, _ = state[b]
                Lvc = min(128, L - c * 128)
                nc.tensor.matmul(qk[0:Lvc, c * G:(c + 1) * G],
                                 KTt[:, c * 128:c * 128 + Lvc], rq,
                                 start=True, stop=True)

            def emit_pv_chunk(b, pv, c, nch2):
                L = Ls[b]
                _, Vt = state[b]
                Lvc = min(128, L - c * 128)
                nc.tensor.matmul(pv[:],
                                 sc_all[0:Lvc, c * B * G + b * G:
                                        c * B * G + (b + 1) * G],
                                 Vt[0:Lvc, c * 128:(c + 1) * 128],
                                 start=(c == 0), stop=(c == nch2 - 1))

            def emit_qk_prologue(b):
                L = Ls[b]
                nch2 = (L + 127) // 128
                Lv = L - (nch2 - 1) * 128
                qk = ps_qk.tile([128, NSUB * G], F32, tag="ps_qk",
                                name=f"qk{b}")
                rq = qT[:].rearrange("p (g b) -> p g b", b=B)[:, :, b]
                if Lv < 128:
                    # pre-fill the tail chunk's block; the matmul then
                    # overwrites rows [0:Lv), leaving -inf beyond the
                    # context (PSUM accesses must be 32-partition-aligned,
                    # so a [Lv:128) memset is not expressible)
                    nc.vector.memset(qk[:, (nch2 - 1) * G:nch2 * G], -1e30)
                return qk, rq, nch2

            def emit_exp(b, qk, nch2):
                scv = sc_all[:].rearrange("p (c w) -> p c w", w=B * G)
                if nch2 < NSUB:
                    nc.vector.memset(
                        scv[:, nch2:NSUB, b * G:(b + 1) * G], 0.0)
                nc.scalar.activation(scv[:, 0:nch2, b * G:(b + 1) * G],
                                     qk[:, 0:nch2 * G],
                                     mybir.ActivationFunctionType.Exp,
                                     scale=SCALE)

            def emit_pv_epilogue(b, pv):
                state.pop(b)
                att = nrmpool.tile([G, HEAD_DIM], BF16, tag="att",
                                   name=f"att{b}")
                nc.vector.tensor_copy(att[:], pv[:])
                nc.gpsimd.dma_start(pvn[G * b:G * (b + 1), :], att[:])

            # software pipeline: QK(b) chunk matmuls interleave with
            # PV(b-1) chunk matmuls so QK's 128-col LDWEIGHTS hides under
            # PV's 128-col moving stream (and vice versa)
            prev = None  # (b, pv_tile, nch)
            for i in range(B):
                b = ORDER[i]
                if i + 3 < B:
                    emit_load(ORDER[i + 3])
                if i in (6, 11, 16, 21):
                    emit_wo_load((i - 6) // 5)
                qk, rq, nch2 = emit_qk_prologue(b)
                if prev is None:
                    for c in range(nch2):
                        emit_qk_chunk(b, qk, rq, c)
                else:
                    pb, pv, pnch = prev
                    for c in range(max(nch2, pnch)):
                        if c < nch2:
                            emit_qk_chunk(b, qk, rq, c)
                        if c < pnch:
                            emit_pv_chunk(pb, pv, c, pnch)
                    emit_pv_epilogue(pb, pv)
                emit_exp(b, qk, nch2)
                pv = ps_pv.tile([G, HEAD_DIM], F32, tag="ps_pv",
                                name=f"pv{b}")
                prev = (b, pv, nch2)
            pb, pv, pnch = prev
            for c in range(pnch):
                emit_pv_chunk(pb, pv, c, pnch)
            emit_pv_epilogue(pb, pv)

        # ---- phase 3: batched softmax denominators + normalize -----------
        with ExitStack() as s4:
            npool = s4.enter_context(tc.tile_pool(name="norm", bufs=1))
            ps_den = s4.enter_context(
                tc.tile_pool(name="ps_den", bufs=1, space="PSUM"))
            ps_t = s4.enter_context(
                tc.tile_pool(name="ps_t", bufs=1, space="PSUM"))
            den_ps = ps_den.tile([128, 1], F32, tag="ps_den")
            for c in range(NSUB):
                nc.tensor.matmul(den_ps[:],
                                 sc_all[:, c * B * G:(c + 1) * B * G],
                                 onesD[:],
                                 start=(c == 0), stop=(c == NSUB - 1))
            rcp = npool.tile([128, 1], F32, tag="rcp")
            nc.vector.reciprocal(rcp[:], den_ps[:])
            pvm = npool.tile([128, 128], BF16, tag="pvm")
            nc.vector.tensor_scalar_mul(pvm[:], pvn[:], rcp[:])
            pvt_ps = ps_t.tile([128, 128], BF16, tag="ps_t")
            nc.tensor.transpose(pvt_ps[:], pvm[:], idb[:])
            nc.vector.tensor_copy(pvT[:], pvt_ps[:])

        # ---- phase 4: output projection ----------------------------------
        with ExitStack() as s5:
            outp = s5.enter_context(tc.tile_pool(name="outp", bufs=1))
            ps_o = s5.enter_context(
                tc.tile_pool(name="ps_o", bufs=8, space="PSUM"))
            out_sb = outp.tile([B, D], F32, tag="out")
            o_ps = [ps_o.tile([B, 512], F32, tag="ps_o", name=f"ops{n}")
                    for n in range(8)]
            pvr = pvT[:].rearrange("p (b g) -> p b g", g=G)
            for g in range(G):
                lt = pvr[:, :, g]
                for n in range(8):
                    nc.tensor.matmul(o_ps[n][:], lt,
                                     wo_tiles[g][:, n * 512:(n + 1) * 512],
                                     start=(g == 0), stop=(g == G - 1))
            for n in range(8):
                if n % 2 == 0:
                    nc.vector.tensor_copy(out_sb[:, n * 512:(n + 1) * 512],
                                          o_ps[n][:])
                else:
                    nc.scalar.copy(out_sb[:, n * 512:(n + 1) * 512],
                                   o_ps[n][:])
            nc.sync.dma_start(out_d[:], out_sb[:])

    nc.compile()
    return nc


def _prep_inputs(x, Wq, Wk, Wv, Wo, key_cache, value_cache, block_tables,
                 context_lens):
    import ml_dtypes
    bf16 = ml_dtypes.bfloat16
    x = np.asarray(x, dtype=np.float32).reshape(B, D)
    # xT[p, kc*32+b] = x[b, kc*128+p]
    xT = np.ascontiguousarray(
        x.reshape(B, 32, 128).transpose(2, 1, 0).reshape(128, 32 * B)
    ).astype(bf16)
    Wq = np.asarray(Wq, dtype=np.float32)
    Wk = np.asarray(Wk, dtype=np.float32)
    Wv = np.asarray(Wv, dtype=np.float32)
    Wo = np.asarray(Wo, dtype=np.float32)
    key_cache = np.asarray(key_cache, dtype=np.float32)
    value_cache = np.asarray(value_cache, dtype=np.float32)
    bt = np.asarray(block_tables, dtype=np.int64)
    cl = np.asarray(context_lens, dtype=np.int64)

    Ls = [int(v) for v in cl]
    pos = np.array([v - 1 for v in Ls], dtype=np.int64)

    # rope tables at the new token's position ([b, half] layout)
    half = HEAD_DIM // 2
    inv_freq = (1.0 / (ROPE_BASE ** (np.arange(half, dtype=np.float32) / half))
                ).astype(np.float32)
    ang = pos.astype(np.float32)[:, None] * inv_freq[None, :]
    cqb = np.ascontiguousarray(np.cos(ang).astype(np.float32))  # [B, 64]
    sqb = np.ascontiguousarray(np.sin(ang).astype(np.float32))
    identb = np.eye(128, dtype=np.float32).astype(bf16)

    # gather runs over t in [0, L-1) - the new token is spliced on device
    runs = [_kv_blocks(bt[b], Ls[b] - 1) for b in range(B)]

    in_maps = []
    for h in range(N_CORES):
        # relaid cache: row p of group g = K^T col d=p / V row t=p
        K = key_cache[:, h, :].reshape(NGRP, 128, HEAD_DIM)
        V = value_cache[:, h, :].reshape(NGRP, 128, HEAD_DIM)
        kt = np.ascontiguousarray(
            K.transpose(2, 0, 1).reshape(128, NGRP * 128)).astype(bf16)
        vv = np.ascontiguousarray(
            V.transpose(1, 0, 2).reshape(128, NGRP * 128)).astype(bf16)
        # fused [Wq | Wk | Wv] moving operand, per 128-row input chunk
        wqkv = np.empty((128, 32 * WQKV), dtype=np.float32)
        for kc in range(32):
            r0 = kc * 128
            wqkv[:, kc * WQKV:kc * WQKV + GD] = \
                Wq[r0:r0 + 128, h * GD:(h + 1) * GD]
            wqkv[:, kc * WQKV + GD:kc * WQKV + GD + HEAD_DIM] = \
                Wk[r0:r0 + 128, h * HEAD_DIM:(h + 1) * HEAD_DIM]
            wqkv[:, kc * WQKV + GD + HEAD_DIM:(kc + 1) * WQKV] = \
                Wv[r0:r0 + 128, h * HEAD_DIM:(h + 1) * HEAD_DIM]
        in_maps.append({
            "xT": xT,
            "wqkv": wqkv.astype(bf16),
            "wo": np.ascontiguousarray(Wo[h * GD:(h + 1) * GD, :]
                                       ).astype(bf16),
            "kt": kt, "vv": vv,
            "cqb": cqb, "sqb": sqb, "identb": identb,
        })
    return Ls, runs, in_maps


def kernel(x, Wq, Wk, Wv, Wo, key_cache, value_cache, block_tables,
           context_lens):
    global LAST_RESULTS
    Ls, runs, in_maps = _prep_inputs(
        x, Wq, Wk, Wv, Wo, key_cache, value_cache, block_tables,
        context_lens)
    nc = _build_nc(Ls, runs)
    res = run_bass_kernel_spmd(nc, in_maps, list(range(N_CORES)))
    LAST_RESULTS = res

    out = np.zeros((B, D), dtype=np.float32)
    for h in range(N_CORES):
        out += res.results[h]["out"]
    return np.ascontiguousarray(out.reshape(B, 1, D))


# revision 11
# speedup vs baseline: 1.0913x; 1.0102x over previous
"""Paged-attention decode kernel for Trainium2 (Bass/Tile), 8 NeuronCores.

Sharding: one KV head per core (N_KV=8). Each core gets x^T plus its head's
slices of the weights and of the paged K/V caches, computes its 4 query
heads' attention and a partial output projection [B, D]; the host sums the
partials.

v3 (vs the f32r baseline):
- everything bf16 on the wire (weights, x, K, V, probabilities): halves HBM
  traffic and removes the f32r 2x LDWEIGHTS / 4x small-moving-matmul
  penalties.
- projections flipped: x^T chunk is the (32-col, cheap) stationary operand,
  fused [Wq|Wk|Wv] is the moving operand -> 64 matmuls instead of 384, and
  no 128-col LDWEIGHTS for weight tiles.
- the new token's k/v are spliced into the gathered K^T/V SBUF tiles (one
  DVE column copy + one SWDGE row copy per request) instead of 4 extra
  matmuls per request.
- softmax denominators for all 32 requests are computed by 17 batched
  matmuls over one shared probability tile (ones-vector contraction) and
  applied as a single [128,128] normalize at the end, instead of per-request
  per-chunk denominator matmuls + 32 reciprocal/mul pairs.
"""
import os
import sys
from contextlib import ExitStack

import numpy as np

for _p in ("/opt/trn_rl_repo", "/opt/pypackages"):
    if os.path.isdir(_p) and _p not in sys.path:
        sys.path.append(_p)

import concourse.bass as bass  # noqa: E402,F401
import concourse.tile as tile  # noqa: E402
from concourse import bacc, mybir  # noqa: E402
from concourse.bass_utils import run_bass_kernel_spmd  # noqa: E402

N_HEADS = 32
N_KV = 8
HEAD_DIM = 128
BLOCK_SIZE = 16
MAX_SEQ = 2048
ROPE_BASE = 10000.0
SCALE = HEAD_DIM ** -0.5
B = 32
D = 4096
G = N_HEADS // N_KV  # 4 query heads per kv head
GD = G * HEAD_DIM    # 512
N_CORES = 8
NGRP = B * MAX_SEQ // 128  # 512 slot groups
MAX_CH = MAX_SEQ // 128    # 16
NSUB = MAX_CH + 1          # score sub-blocks incl. tail slack: 17
WQKV = GD + 2 * HEAD_DIM   # 768 fused projection cols per input chunk

F32 = mybir.dt.float32
BF16 = mybir.dt.bfloat16

LAST_RESULTS = None  # test harness reads exec_time_ns from here


def _kv_blocks(bt_row, L):
    """16-slot gather blocks [(slot_start, t_start, n_rows)] covering
    t in [0, L), runs coalesced."""
    nblocks = (L + BLOCK_SIZE - 1) // BLOCK_SIZE
    runs = []
    cur_s = cur_t = cur_n = 0
    for j in range(nblocks):
        rows = min(BLOCK_SIZE, L - j * BLOCK_SIZE)
        s = int(bt_row[j]) * BLOCK_SIZE
        if cur_n and s == cur_s + cur_n:
            cur_n += rows
        else:
            if cur_n:
                runs.append((cur_s, cur_t, cur_n))
            cur_s, cur_t, cur_n = s, j * BLOCK_SIZE, rows
    if cur_n:
        runs.append((cur_s, cur_t, cur_n))
    return runs


def _emit_kv_dmas(engs, kt_d, vv_d, KTt, Vt, runs, ei=0):
    """Gather the relaid caches: K^T (partition=d) and V (partition=slot%128)
    for chunk c land at cols [c*128, (c+1)*128)."""
    def dma(dst, srca):
        nonlocal ei
        engs[ei % len(engs)].dma_start(dst, srca)
        ei += 1

    for (s0, t0, n) in runs:
        if (s0 % 128 == 0 and t0 % 128 == 0
                and (s0 + n + 127) // 128 <= NGRP):
            # contiguous aligned run: K as one flat DMA; V rounded up to
            # whole 128-slot groups (over-read stays in-bounds, and any
            # garbage tail rows sit beyond Lv so they are never read)
            nf = (n + 127) // 128
            g0, c0 = s0 // 128, t0 // 128
            dma(KTt[:, t0:t0 + n], kt_d[:, s0:s0 + n])
            dma(Vt[:, c0 * 128:(c0 + nf) * 128],
                vv_d[:, g0 * 128:(g0 + nf) * 128])
            continue
        while n > 0:
            if s0 % 128 == 0 and t0 % 128 == 0 and n >= 128:
                nfull = n // 128
                g0, c0 = s0 // 128, t0 // 128
                dma(KTt[:, c0 * 128:(c0 + nfull) * 128],
                    kt_d[:, g0 * 128:(g0 + nfull) * 128])
                dma(Vt[:, c0 * 128:(c0 + nfull) * 128],
                    vv_d[:, g0 * 128:(g0 + nfull) * 128])
                take = nfull * 128
            else:
                g0, o = s0 // 128, s0 % 128
                c0, to = t0 // 128, t0 % 128
                take = min(n, 128 - o, 128 - to)
                dma(KTt[:, c0 * 128 + to:c0 * 128 + to + take],
                    kt_d[:, g0 * 128 + o:g0 * 128 + o + take])
                dma(Vt[to:to + take, c0 * 128:(c0 + 1) * 128],
                    vv_d[o:o + take, g0 * 128:(g0 + 1) * 128])
            s0 += take
            t0 += take
            n -= take
    return ei


def _build_nc(Ls, runs_all):
    ORDER = sorted(range(B), key=lambda b: -Ls[b])
    nc = bacc.Bacc("TRN2", target_bir_lowering=False, debug=False,
                   num_devices=N_CORES)

    xt_d = nc.declare_dram_parameter("xT", [128, 32 * B], BF16, isOutput=False)
    wqkv_d = nc.declare_dram_parameter("wqkv", [128, 32 * WQKV], BF16,
                                       isOutput=False)
    wo_d = nc.declare_dram_parameter("wo", [GD, D], BF16, isOutput=False)
    kt_d = nc.declare_dram_parameter("kt", [128, NGRP * 128], BF16,
                                     isOutput=False)
    vv_d = nc.declare_dram_parameter("vv", [128, NGRP * 128], BF16,
                                     isOutput=False)
    cqb_d = nc.declare_dram_parameter("cqb", [B, G * 64], F32,
                                  isOutput=False)
    sqb_d = nc.declare_dram_parameter("sqb", [B, G * 64], F32,
                                      isOutput=False)
    idb_d = nc.declare_dram_parameter("identb", [128, 128], BF16,
                                      isOutput=False)
    out_d = nc.declare_dram_parameter("out", [B, D], F32, isOutput=True)

    with tile.TileContext(nc) as tc, ExitStack() as top:
        cpool = top.enter_context(tc.tile_pool(name="const", bufs=1))
        qT = cpool.tile([128, G * B], BF16, tag="qT")    # [d, g*32+b] roped
        knT = cpool.tile([128, B], BF16, tag="knT")      # [d, b] roped new k
        vn = cpool.tile([B, HEAD_DIM], BF16, tag="vn")   # [b, d] new v
        pvn = cpool.tile([128, 128], BF16, tag="pvn")    # raw pv [b*4+g, d]
        pvT = cpool.tile([128, 128], BF16, tag="pvT")    # [d, b*4+g]
        sc_all = cpool.tile([128, B * NSUB * G], BF16, tag="SC")
        onesD = cpool.tile([128, 1], BF16, tag="ones")
        nc.vector.memset(onesD[:], 1.0)
        idb = cpool.tile([128, 128], BF16, tag="identb")
        nc.sync.dma_start(idb[:], idb_d[:])

        kvpool = top.enter_context(tc.tile_pool(name="KV", bufs=6))
        nrmpool = top.enter_context(tc.tile_pool(name="nrm", bufs=4))
        wop = top.enter_context(tc.tile_pool(name="wo", bufs=4))
        state = {}
        wo_tiles = []
        dma_rr = [0]

        def emit_splice(b):
            # splice the new token's k/v into the gathered tiles, well
            # ahead of the QK/PV matmuls that read them
            KTt, Vt = state[b]
            Lg = Ls[b] - 1
            nc.vector.tensor_copy(KTt[:, Lg:Lg + 1], knT[:, b:b + 1])
            nc.gpsimd.dma_start(
                Vt[Lg % 128:Lg % 128 + 1,
                   (Lg // 128) * 128:(Lg // 128 + 1) * 128],
                vn[b:b + 1, :])

        def emit_load(b, splice=True):
            KTt = kvpool.tile([128, MAX_CH * 128], BF16, tag="KT",
                              name=f"KTt{b}")
            Vt = kvpool.tile([128, MAX_CH * 128], BF16, tag="V",
                             name=f"Vt{b}")
            dma_rr[0] = _emit_kv_dmas([nc.sync, nc.scalar], kt_d, vv_d,
                                      KTt, Vt, runs_all[b], dma_rr[0])
            state[b] = (KTt, Vt)
            if splice:
                emit_splice(b)

        def emit_wo_load(g):
            wo_t = wop.tile([128, D], BF16, tag="wo", name=f"wo{g}")
            eng = nc.sync if g % 2 == 0 else nc.scalar
            eng.dma_start(wo_t[:], wo_d[g * 128:(g + 1) * 128, :])
            wo_tiles.append(wo_t)

        # ---- phase 1: fused q/k/v projection (x^T stationary) + rope -----
        with ExitStack() as s1:
            p1 = s1.enter_context(tc.tile_pool(name="p1", bufs=1))
            wqp = s1.enter_context(tc.tile_pool(name="wqp", bufs=8))
            ps_q = s1.enter_context(
                tc.tile_pool(name="ps_q", bufs=1, space="PSUM"))
            ps_kv = s1.enter_context(
                tc.tile_pool(name="ps_kv", bufs=1, space="PSUM"))
            ps_tr = s1.enter_context(
                tc.tile_pool(name="ps_tr", bufs=2, space="PSUM"))
            tmp = s1.enter_context(tc.tile_pool(name="rtmp", bufs=4))

            xT = p1.tile([128, 32 * B], BF16, tag="xT")   # [d, kc*32+b]
            nc.sync.dma_start(xT[:], xt_d[:])
            cqb = p1.tile([B, G * 64], F32, tag="cqb")
            sqb = p1.tile([B, G * 64], F32, tag="sqb")
            nc.scalar.dma_start(cqb[:], cqb_d[:])
            nc.scalar.dma_start(sqb[:], sqb_d[:])

            q_ps = ps_q.tile([B, GD], F32, tag="ps_q")
            kv_ps = ps_kv.tile([B, 2 * HEAD_DIM], F32, tag="ps_kv")
            # warm-up: keep the PE busy while the first weight chunk is in
            # flight so the clock p-state ramps before the real matmuls
            with ExitStack() as sw:
                wpool = sw.enter_context(tc.tile_pool(name="warm", bufs=1))
                ps_w = sw.enter_context(
                    tc.tile_pool(name="ps_w", bufs=1, space="PSUM"))
                wsrc = wpool.tile([128, 128], BF16, tag="wsrc")
                nc.vector.memset(wsrc[:], 0.0)
                w_ps = ps_w.tile([128, 128], F32, tag="ps_w")
                for _ in range(40):
                    nc.tensor.matmul(w_ps[:], wsrc[:], wsrc[:],
                                     start=True, stop=True)
            for j in range(8):
                wqkv_t = wqp.tile([128, 4 * WQKV], BF16, tag="wqkv",
                                  name=f"wqkv{j}")
                eng = nc.sync if j % 2 == 0 else nc.scalar
                eng.dma_start(wqkv_t[:],
                              wqkv_d[:, j * 4 * WQKV:(j + 1) * 4 * WQKV])
                for ci in range(4):
                    kc = j * 4 + ci
                    lx = xT[:, kc * B:(kc + 1) * B]
                    nc.tensor.matmul(q_ps[:],
                                     lx, wqkv_t[:, ci * WQKV:ci * WQKV + GD],
                                     start=(kc == 0), stop=(kc == 31))
                    nc.tensor.matmul(
                        kv_ps[:], lx,
                        wqkv_t[:, ci * WQKV + GD:(ci + 1) * WQKV],
                        start=(kc == 0), stop=(kc == 31))

            emit_load(ORDER[0], splice=False)
            emit_load(ORDER[1], splice=False)
            emit_load(ORDER[2], splice=False)

            # rope in [b, d] layout: halves d<64 / d>=64, then transpose
            qro = p1.tile([B, GD], BF16, tag="qro")
            kro = p1.tile([B, HEAD_DIM], BF16, tag="kro")

            def rope_b(src, out, w):
                # src/out [B, w*128] viewed [B, w, {lo,hi} 64]; tables
                # [B, w*64] (host-repeated): 6 wide DVE ops total
                sv = src.rearrange("b (w h) -> b w h", h=128)
                ov = out.rearrange("b (w h) -> b w h", h=128)
                lo_s, hi_s = sv[:, :, 0:64], sv[:, :, 64:128]
                lo_o, hi_o = ov[:, :, 0:64], ov[:, :, 64:128]
                cw = cqb[:, 0:w * 64].rearrange("b (w h) -> b w h", h=64)
                sw = sqb[:, 0:w * 64].rearrange("b (w h) -> b w h", h=64)
                t1 = tmp.tile([B, w * 64], F32, tag="rt1", name=f"t1_{w}")
                t2 = tmp.tile([B, w * 64], F32, tag="rt2", name=f"t2_{w}")
                t1v = t1[:].rearrange("b (w h) -> b w h", h=64)
                t2v = t2[:].rearrange("b (w h) -> b w h", h=64)
                nc.vector.tensor_mul(t1v, lo_s, cw)
                nc.vector.tensor_mul(t2v, hi_s, sw)
                nc.vector.tensor_sub(lo_o, t1v, t2v)
                nc.vector.tensor_mul(t1v, lo_s, sw)
                nc.vector.tensor_mul(t2v, hi_s, cw)
                nc.vector.tensor_add(hi_o, t1v, t2v)

            rope_b(q_ps[:], qro[:], G)
            rope_b(kv_ps[:, 0:128], kro[:], 1)
            nc.vector.tensor_copy(vn[:], kv_ps[:, 128:256])

            for g in range(G):
                trq = ps_tr.tile([128, B], BF16, tag="ps_tr", name=f"trq{g}")
                nc.tensor.transpose(trq[:], qro[:, g * 128:(g + 1) * 128],
                                    idb[0:B, 0:B])
                nc.vector.tensor_copy(qT[:, g * B:(g + 1) * B], trq[:])
            trk = ps_tr.tile([128, B], BF16, tag="ps_tr", name="trk")
            nc.tensor.transpose(trk[:], kro[:], idb[0:B, 0:B])
            nc.vector.tensor_copy(knT[:], trk[:])
            for i in range(3):
                emit_splice(ORDER[i])

        # ---- phase 2: per-request attention ------------------------------
        with ExitStack() as s3:
            ps_qk = s3.enter_context(
                tc.tile_pool(name="ps_qk", bufs=3, space="PSUM"))
            ps_pv = s3.enter_context(
                tc.tile_pool(name="ps_pv", bufs=2, space="PSUM"))

            def emit_qk_chunk(b, qk, rq, c):
                L = Ls[b]
                KTt, _ = state[b]
                Lvc = min(128, L - c * 128)
                nc.tensor.matmul(qk[0:Lvc, c * G:(c + 1) * G],
                                 KTt[:, c * 128:c * 128 + Lvc], rq,
                                 start=True, stop=True)

            def emit_pv_chunk(b, pv, c, nch2):
                L = Ls[b]
                _, Vt = state[b]
                Lvc = min(128, L - c * 128)
                nc.tensor.matmul(pv[:],
                                 sc_all[0:Lvc, c * B * G + b * G:
                                        c * B * G + (b + 1) * G],
                                 Vt[0:Lvc, c * 128:(c + 1) * 128],
                                 start=(c == 0), stop=(c == nch2 - 1))

            def emit_qk_prologue(b):
                L = Ls[b]
                nch2 = (L + 127) // 128
                Lv = L - (nch2 - 1) * 128
                qk = ps_qk.tile([128, NSUB * G], F32, tag="ps_qk",
                                name=f"qk{b}")
                rq = qT[:].rearrange("p (g b) -> p g b", b=B)[:, :, b]
                if Lv < 128:
                    # pre-fill the tail chunk's block; the matmul then
                    # overwrites rows [0:Lv), leaving -inf beyond the
                    # context (PSUM accesses must be 32-partition-aligned,
                    # so a [Lv:128) memset is not expressible)
                    nc.vector.memset(qk[:, (nch2 - 1) * G:nch2 * G], -1e30)
                return qk, rq, nch2

            def emit_exp(b, qk, nch2):
                scv = sc_all[:].rearrange("p (c w) -> p c w", w=B * G)
                if nch2 < NSUB:
                    nc.vector.memset(
                        scv[:, nch2:NSUB, b * G:(b + 1) * G], 0.0)
                nc.scalar.activation(scv[:, 0:nch2, b * G:(b + 1) * G],
                                     qk[:, 0:nch2 * G],
                                     mybir.ActivationFunctionType.Exp,
                                     scale=SCALE)

            def emit_pv_epilogue(b, pv, last=False):
                state.pop(b)
                att = nrmpool.tile([G, HEAD_DIM], BF16, tag="att",
                                   name=f"att{b}")
                nc.vector.tensor_copy(att[:], pv[:])
                eng = nc.sync if last else nc.gpsimd
                eng.dma_start(pvn[G * b:G * (b + 1), :], att[:])

            # software pipeline: QK(b) chunk matmuls interleave with
            # PV(b-1) chunk matmuls so QK's 128-col LDWEIGHTS hides under
            # PV's 128-col moving stream (and vice versa)
            prev = None  # (b, pv_tile, nch)
            for i in range(B):
                b = ORDER[i]
                if i + 3 < B:
                    emit_load(ORDER[i + 3])
                if i in (6, 11, 16, 21):
                    emit_wo_load((i - 6) // 5)
                qk, rq, nch2 = emit_qk_prologue(b)
                if prev is None:
                    for c in range(nch2):
                        emit_qk_chunk(b, qk, rq, c)
                else:
                    pb, pv, pnch = prev
                    for c in range(max(nch2, pnch)):
                        if c < nch2:
                            emit_qk_chunk(b, qk, rq, c)
                        if c < pnch:
                            emit_pv_chunk(pb, pv, c, pnch)
                    emit_pv_epilogue(pb, pv)
                emit_exp(b, qk, nch2)
                pv = ps_pv.tile([G, HEAD_DIM], F32, tag="ps_pv",
                                name=f"pv{b}")
                prev = (b, pv, nch2)
            pb, pv, pnch = prev
            for c in range(pnch):
                emit_pv_chunk(pb, pv, c, pnch)
            emit_pv_epilogue(pb, pv, last=True)

        # ---- phase 3: batched softmax denominators + normalize -----------
        with ExitStack() as s4:
            npool = s4.enter_context(tc.tile_pool(name="norm", bufs=1))
            ps_den = s4.enter_context(
                tc.tile_pool(name="ps_den", bufs=1, space="PSUM"))
            ps_t = s4.enter_context(
                tc.tile_pool(name="ps_t", bufs=1, space="PSUM"))
            den_ps = ps_den.tile([128, 1], F32, tag="ps_den")
            for c in range(NSUB):
                nc.tensor.matmul(den_ps[:],
                                 sc_all[:, c * B * G:(c + 1) * B * G],
                                 onesD[:],
                                 start=(c == 0), stop=(c == NSUB - 1))
            rcp = npool.tile([128, 1], F32, tag="rcp")
            nc.vector.reciprocal(rcp[:], den_ps[:])
            pvm = npool.tile([128, 128], BF16, tag="pvm")
            nc.vector.tensor_scalar_mul(pvm[:], pvn[:], rcp[:])
            pvt_ps = ps_t.tile([128, 128], BF16, tag="ps_t")
            nc.tensor.transpose(pvt_ps[:], pvm[:], idb[:])
            nc.vector.tensor_copy(pvT[:], pvt_ps[:])

        # ---- phase 4: output projection ----------------------------------
        with ExitStack() as s5:
            outp = s5.enter_context(tc.tile_pool(name="outp", bufs=1))
            ps_o = s5.enter_context(
                tc.tile_pool(name="ps_o", bufs=8, space="PSUM"))
            out_sb = outp.tile([B, D], F32, tag="out")
            o_ps = [ps_o.tile([B, 512], F32, tag="ps_o", name=f"ops{n}")
                    for n in range(8)]
            pvr = pvT[:].rearrange("p (b g) -> p b g", g=G)
            for g in range(G):
                lt = pvr[:, :, g]
                for n in range(8):
                    nc.tensor.matmul(o_ps[n][:], lt,
                                     wo_tiles[g][:, n * 512:(n + 1) * 512],
                                     start=(g == 0), stop=(g == G - 1))
            for n in range(8):
                if n % 2 == 0:
                    nc.vector.tensor_copy(out_sb[:, n * 512:(n + 1) * 512],
                                          o_ps[n][:])
                else:
                    nc.scalar.copy(out_sb[:, n * 512:(n + 1) * 512],
                                   o_ps[n][:])
            nc.sync.dma_start(out_d[:], out_sb[:])

    nc.compile()
    return nc


def _prep_inputs(x, Wq, Wk, Wv, Wo, key_cache, value_cache, block_tables,
                 context_lens):
    import ml_dtypes
    bf16 = ml_dtypes.bfloat16
    x = np.asarray(x, dtype=np.float32).reshape(B, D)
    # xT[p, kc*32+b] = x[b, kc*128+p]
    xT = np.ascontiguousarray(
        x.reshape(B, 32, 128).transpose(2, 1, 0).reshape(128, 32 * B)
    ).astype(bf16)
    Wq = np.asarray(Wq, dtype=np.float32)
    Wk = np.asarray(Wk, dtype=np.float32)
    Wv = np.asarray(Wv, dtype=np.float32)
    Wo = np.asarray(Wo, dtype=np.float32)
    key_cache = np.asarray(key_cache, dtype=np.float32)
    value_cache = np.asarray(value_cache, dtype=np.float32)
    bt = np.asarray(block_tables, dtype=np.int64)
    cl = np.asarray(context_lens, dtype=np.int64)

    Ls = [int(v) for v in cl]
    pos = np.array([v - 1 for v in Ls], dtype=np.int64)

    # rope tables at the new token's position ([b, half] layout)
    half = HEAD_DIM // 2
    inv_freq = (1.0 / (ROPE_BASE ** (np.arange(half, dtype=np.float32) / half))
                ).astype(np.float32)
    ang = pos.astype(np.float32)[:, None] * inv_freq[None, :]
    cqb = np.ascontiguousarray(
        np.tile(np.cos(ang).astype(np.float32), (1, G)))  # [B, 4*64]
    sqb = np.ascontiguousarray(
        np.tile(np.sin(ang).astype(np.float32), (1, G)))
    identb = np.eye(128, dtype=np.float32).astype(bf16)

    # gather runs over t in [0, L-1) - the new token is spliced on device
    runs = [_kv_blocks(bt[b], Ls[b] - 1) for b in range(B)]

    in_maps = []
    for h in range(N_CORES):
        # relaid cache: row p of group g = K^T col d=p / V row t=p
        K = key_cache[:, h, :].reshape(NGRP, 128, HEAD_DIM)
        V = value_cache[:, h, :].reshape(NGRP, 128, HEAD_DIM)
        kt = np.ascontiguousarray(
            K.transpose(2, 0, 1).reshape(128, NGRP * 128)).astype(bf16)
        vv = np.ascontiguousarray(
            V.transpose(1, 0, 2).reshape(128, NGRP * 128)).astype(bf16)
        # fused [Wq | Wk | Wv] moving operand, per 128-row input chunk
        wqkv = np.empty((128, 32 * WQKV), dtype=np.float32)
        for kc in range(32):
            r0 = kc * 128
            wqkv[:, kc * WQKV:kc * WQKV + GD] = \
                Wq[r0:r0 + 128, h * GD:(h + 1) * GD]
            wqkv[:, kc * WQKV + GD:kc * WQKV + GD + HEAD_DIM] = \
                Wk[r0:r0 + 128, h * HEAD_DIM:(h + 1) * HEAD_DIM]
            wqkv[:, kc * WQKV + GD + HEAD_DIM:(kc + 1) * WQKV] = \
                Wv[r0:r0 + 128, h * HEAD_DIM:(h + 1) * HEAD_DIM]
        in_maps.append({
            "xT": xT,
            "wqkv": wqkv.astype(bf16),
            "wo": np.ascontiguousarray(Wo[h * GD:(h + 1) * GD, :]
                                       ).astype(bf16),
            "kt": kt, "vv": vv,
            "cqb": cqb, "sqb": sqb, "identb": identb,
        })
    return Ls, runs, in_maps


def kernel(x, Wq, Wk, Wv, Wo, key_cache, value_cache, block_tables,
           context_lens):
    global LAST_RESULTS
    Ls, runs, in_maps = _prep_inputs(
        x, Wq, Wk, Wv, Wo, key_cache, value_cache, block_tables,
        context_lens)
    nc = _build_nc(Ls, runs)
    res = run_bass_kernel_spmd(nc, in_maps, list(range(N_CORES)))
    LAST_RESULTS = res

    out = np.zeros((B, D), dtype=np.float32)
    for h in range(N_CORES):
        out += res.results[h]["out"]
    return np.ascontiguousarray(out.reshape(B, 1, D))


# revision 12
# speedup vs baseline: 1.1222x; 1.0283x over previous
"""Paged-attention decode kernel for Trainium2 (Bass/Tile), 8 NeuronCores.

Sharding: one KV head per core (N_KV=8). Each core gets x^T plus its head's
slices of the weights and of the paged K/V caches, computes its 4 query
heads' attention and a partial output projection [B, D]; the host sums the
partials.

v3 (vs the f32r baseline):
- everything bf16 on the wire (weights, x, K, V, probabilities): halves HBM
  traffic and removes the f32r 2x LDWEIGHTS / 4x small-moving-matmul
  penalties.
- projections flipped: x^T chunk is the (32-col, cheap) stationary operand,
  fused [Wq|Wk|Wv] is the moving operand -> 64 matmuls instead of 384, and
  no 128-col LDWEIGHTS for weight tiles.
- the new token's k/v are spliced into the gathered K^T/V SBUF tiles (one
  DVE column copy + one SWDGE row copy per request) instead of 4 extra
  matmuls per request.
- softmax denominators for all 32 requests are computed by 17 batched
  matmuls over one shared probability tile (ones-vector contraction) and
  applied as a single [128,128] normalize at the end, instead of per-request
  per-chunk denominator matmuls + 32 reciprocal/mul pairs.
"""
import os
import sys
from contextlib import ExitStack

import numpy as np

for _p in ("/opt/trn_rl_repo", "/opt/pypackages"):
    if os.path.isdir(_p) and _p not in sys.path:
        sys.path.append(_p)

import concourse.bass as bass  # noqa: E402,F401
import concourse.tile as tile  # noqa: E402
from concourse import bacc, mybir  # noqa: E402
from concourse.bass_utils import run_bass_kernel_spmd  # noqa: E402

N_HEADS = 32
N_KV = 8
HEAD_DIM = 128
BLOCK_SIZE = 16
MAX_SEQ = 2048
ROPE_BASE = 10000.0
SCALE = HEAD_DIM ** -0.5
B = 32
D = 4096
G = N_HEADS // N_KV  # 4 query heads per kv head
GD = G * HEAD_DIM    # 512
N_CORES = 8
NGRP = B * MAX_SEQ // 128  # 512 slot groups
MAX_CH = MAX_SEQ // 128    # 16
NSUB = MAX_CH + 1          # score sub-blocks incl. tail slack: 17
WQKV = GD + 2 * HEAD_DIM   # 768 fused projection cols per input chunk

F32 = mybir.dt.float32
BF16 = mybir.dt.bfloat16

LAST_RESULTS = None  # test harness reads exec_time_ns from here


def _kv_blocks(bt_row, L):
    """16-slot gather blocks [(slot_start, t_start, n_rows)] covering
    t in [0, L), runs coalesced."""
    nblocks = (L + BLOCK_SIZE - 1) // BLOCK_SIZE
    runs = []
    cur_s = cur_t = cur_n = 0
    for j in range(nblocks):
        rows = min(BLOCK_SIZE, L - j * BLOCK_SIZE)
        s = int(bt_row[j]) * BLOCK_SIZE
        if cur_n and s == cur_s + cur_n:
            cur_n += rows
        else:
            if cur_n:
                runs.append((cur_s, cur_t, cur_n))
            cur_s, cur_t, cur_n = s, j * BLOCK_SIZE, rows
    if cur_n:
        runs.append((cur_s, cur_t, cur_n))
    return runs


def _emit_kv_dmas(engs, kt_d, vv_d, KTt, Vt, runs, ei=0):
    """Gather the relaid caches: K^T (partition=d) and V (partition=slot%128)
    for chunk c land at cols [c*128, (c+1)*128)."""
    def dma(dst, srca):
        nonlocal ei
        engs[ei % len(engs)].dma_start(dst, srca)
        ei += 1

    for (s0, t0, n) in runs:
        if (s0 % 128 == 0 and t0 % 128 == 0
                and (s0 + n + 127) // 128 <= NGRP):
            # contiguous aligned run: K as one flat DMA; V rounded up to
            # whole 128-slot groups (over-read stays in-bounds, and any
            # garbage tail rows sit beyond Lv so they are never read)
            nf = (n + 127) // 128
            g0, c0 = s0 // 128, t0 // 128
            dma(KTt[:, t0:t0 + n], kt_d[:, s0:s0 + n])
            dma(Vt[:, c0 * 128:(c0 + nf) * 128],
                vv_d[:, g0 * 128:(g0 + nf) * 128])
            continue
        while n > 0:
            if s0 % 128 == 0 and t0 % 128 == 0 and n >= 128:
                nfull = n // 128
                g0, c0 = s0 // 128, t0 // 128
                dma(KTt[:, c0 * 128:(c0 + nfull) * 128],
                    kt_d[:, g0 * 128:(g0 + nfull) * 128])
                dma(Vt[:, c0 * 128:(c0 + nfull) * 128],
                    vv_d[:, g0 * 128:(g0 + nfull) * 128])
                take = nfull * 128
            else:
                g0, o = s0 // 128, s0 % 128
                c0, to = t0 // 128, t0 % 128
                take = min(n, 128 - o, 128 - to)
                dma(KTt[:, c0 * 128 + to:c0 * 128 + to + take],
                    kt_d[:, g0 * 128 + o:g0 * 128 + o + take])
                dma(Vt[to:to + take, c0 * 128:(c0 + 1) * 128],
                    vv_d[o:o + take, g0 * 128:(g0 + 1) * 128])
            s0 += take
            t0 += take
            n -= take
    return ei


def _build_nc(Ls, runs_all):
    ORDER = sorted(range(B), key=lambda b: -Ls[b])
    nc = bacc.Bacc("TRN2", target_bir_lowering=False, debug=False,
                   num_devices=N_CORES)

    xt_d = nc.declare_dram_parameter("xT", [128, 32 * B], BF16, isOutput=False)
    wqkv_d = nc.declare_dram_parameter("wqkv", [128, 32 * WQKV], BF16,
                                       isOutput=False)
    wo_d = nc.declare_dram_parameter("wo", [GD, D], BF16, isOutput=False)
    kt_d = nc.declare_dram_parameter("kt", [128, NGRP * 128], BF16,
                                     isOutput=False)
    vv_d = nc.declare_dram_parameter("vv", [128, NGRP * 128], BF16,
                                     isOutput=False)
    cqb_d = nc.declare_dram_parameter("cqb", [B, G * 64], F32,
                                  isOutput=False)
    sqb_d = nc.declare_dram_parameter("sqb", [B, G * 64], F32,
                                      isOutput=False)
    idb_d = nc.declare_dram_parameter("identb", [128, 128], BF16,
                                      isOutput=False)
    out_d = nc.declare_dram_parameter("out", [B, D], F32, isOutput=True)

    with tile.TileContext(nc) as tc, ExitStack() as top:
        cpool = top.enter_context(tc.tile_pool(name="const", bufs=1))
        qT = cpool.tile([128, G * B], BF16, tag="qT")    # [d, g*32+b] roped
        knT = cpool.tile([128, B], BF16, tag="knT")      # [d, b] roped new k
        vn = cpool.tile([B, HEAD_DIM], BF16, tag="vn")   # [b, d] new v
        pvn = cpool.tile([128, 128], BF16, tag="pvn")    # raw pv [b*4+g, d]
        pvT = cpool.tile([128, 128], BF16, tag="pvT")    # [d, b*4+g]
        sc_all = cpool.tile([128, B * NSUB * G], BF16, tag="SC")
        onesD = cpool.tile([128, 1], BF16, tag="ones")
        nc.vector.memset(onesD[:], 1.0)
        idb = cpool.tile([128, 128], BF16, tag="identb")

        kvpool = top.enter_context(tc.tile_pool(name="KV", bufs=8))
        nrmpool = top.enter_context(tc.tile_pool(name="nrm", bufs=4))
        wop = top.enter_context(tc.tile_pool(name="wo", bufs=4))
        state = {}
        wo_tiles = []
        dma_rr = [0]

        def emit_splice(b):
            # splice the new token's k/v into the gathered tiles, well
            # ahead of the QK/PV matmuls that read them
            KTt, Vt = state[b]
            Lg = Ls[b] - 1
            nc.vector.tensor_copy(KTt[:, Lg:Lg + 1], knT[:, b:b + 1])
            nc.gpsimd.dma_start(
                Vt[Lg % 128:Lg % 128 + 1,
                   (Lg // 128) * 128:(Lg // 128 + 1) * 128],
                vn[b:b + 1, :])

        def emit_load(b, splice=True):
            KTt = kvpool.tile([128, MAX_CH * 128], BF16, tag="KT",
                              name=f"KTt{b}")
            Vt = kvpool.tile([128, MAX_CH * 128], BF16, tag="V",
                             name=f"Vt{b}")
            dma_rr[0] = _emit_kv_dmas([nc.sync, nc.scalar], kt_d, vv_d,
                                      KTt, Vt, runs_all[b], dma_rr[0])
            state[b] = (KTt, Vt)
            if splice:
                emit_splice(b)

        def emit_wo_load(g):
            wo_t = wop.tile([128, D], BF16, tag="wo", name=f"wo{g}")
            eng = nc.sync if g % 2 == 0 else nc.scalar
            eng.dma_start(wo_t[:], wo_d[g * 128:(g + 1) * 128, :])
            wo_tiles.append(wo_t)

        # ---- phase 1: fused q/k/v projection (x^T stationary) + rope -----
        with ExitStack() as s1:
            p1 = s1.enter_context(tc.tile_pool(name="p1", bufs=1))
            wqp = s1.enter_context(tc.tile_pool(name="wqp", bufs=4))
            ps_q = s1.enter_context(
                tc.tile_pool(name="ps_q", bufs=1, space="PSUM"))
            ps_kv = s1.enter_context(
                tc.tile_pool(name="ps_kv", bufs=1, space="PSUM"))
            ps_tr = s1.enter_context(
                tc.tile_pool(name="ps_tr", bufs=2, space="PSUM"))
            tmp = s1.enter_context(tc.tile_pool(name="rtmp", bufs=4))

            xT = p1.tile([128, 32 * B], BF16, tag="xT")   # [d, kc*32+b]
            cqb = p1.tile([B, G * 64], F32, tag="cqb")
            sqb = p1.tile([B, G * 64], F32, tag="sqb")
            nc.scalar.dma_start(xT[:], xt_d[:])
            nc.scalar.dma_start(cqb[:], cqb_d[:])
            nc.scalar.dma_start(sqb[:], sqb_d[:])
            nc.scalar.dma_start(idb[:], idb_d[:])

            q_ps = ps_q.tile([B, GD], F32, tag="ps_q")
            kv_ps = ps_kv.tile([B, 2 * HEAD_DIM], F32, tag="ps_kv")
            # warm-up: keep the PE busy while the first weight chunk is in
            # flight so the clock p-state ramps before the real matmuls
            with ExitStack() as sw:
                wpool = sw.enter_context(tc.tile_pool(name="warm", bufs=1))
                ps_w = sw.enter_context(
                    tc.tile_pool(name="ps_w", bufs=1, space="PSUM"))
                wsrc = wpool.tile([128, 128], BF16, tag="wsrc")
                nc.vector.memset(wsrc[:], 0.0)
                w_ps = ps_w.tile([128, 128], F32, tag="ps_w")
                for _ in range(40):
                    nc.tensor.matmul(w_ps[:], wsrc[:], wsrc[:],
                                     start=True, stop=True)
            for j in range(4):
                wqkv_t = wqp.tile([128, 8 * WQKV], BF16, tag="wqkv",
                                  name=f"wqkv{j}")
                nc.sync.dma_start(wqkv_t[:],
                                  wqkv_d[:, j * 8 * WQKV:(j + 1) * 8 * WQKV])
                for ci in range(8):
                    kc = j * 8 + ci
                    lx = xT[:, kc * B:(kc + 1) * B]
                    nc.tensor.matmul(q_ps[:],
                                     lx, wqkv_t[:, ci * WQKV:ci * WQKV + GD],
                                     start=(kc == 0), stop=(kc == 31))
                    nc.tensor.matmul(
                        kv_ps[:], lx,
                        wqkv_t[:, ci * WQKV + GD:(ci + 1) * WQKV],
                        start=(kc == 0), stop=(kc == 31))

            emit_load(ORDER[0], splice=False)
            emit_load(ORDER[1], splice=False)
            emit_load(ORDER[2], splice=False)
            emit_load(ORDER[3], splice=False)

            # rope in [b, d] layout: halves d<64 / d>=64, then transpose
            qro = p1.tile([B, GD], BF16, tag="qro")
            kro = p1.tile([B, HEAD_DIM], BF16, tag="kro")

            def rope_b(src, out, w):
                # src/out [B, w*128] viewed [B, w, {lo,hi} 64]; tables
                # [B, w*64] (host-repeated): 6 wide DVE ops total
                sv = src.rearrange("b (w h) -> b w h", h=128)
                ov = out.rearrange("b (w h) -> b w h", h=128)
                lo_s, hi_s = sv[:, :, 0:64], sv[:, :, 64:128]
                lo_o, hi_o = ov[:, :, 0:64], ov[:, :, 64:128]
                cw = cqb[:, 0:w * 64].rearrange("b (w h) -> b w h", h=64)
                sw = sqb[:, 0:w * 64].rearrange("b (w h) -> b w h", h=64)
                t1 = tmp.tile([B, w * 64], F32, tag="rt1", name=f"t1_{w}")
                t2 = tmp.tile([B, w * 64], F32, tag="rt2", name=f"t2_{w}")
                t1v = t1[:].rearrange("b (w h) -> b w h", h=64)
                t2v = t2[:].rearrange("b (w h) -> b w h", h=64)
                nc.vector.tensor_mul(t1v, lo_s, cw)
                nc.vector.tensor_mul(t2v, hi_s, sw)
                nc.vector.tensor_sub(lo_o, t1v, t2v)
                nc.vector.tensor_mul(t1v, lo_s, sw)
                nc.vector.tensor_mul(t2v, hi_s, cw)
                nc.vector.tensor_add(hi_o, t1v, t2v)

            rope_b(q_ps[:], qro[:], G)
            rope_b(kv_ps[:, 0:128], kro[:], 1)
            nc.vector.tensor_copy(vn[:], kv_ps[:, 128:256])

            for g in range(G):
                trq = ps_tr.tile([128, B], BF16, tag="ps_tr", name=f"trq{g}")
                nc.tensor.transpose(trq[:], qro[:, g * 128:(g + 1) * 128],
                                    idb[0:B, 0:B])
                nc.vector.tensor_copy(qT[:, g * B:(g + 1) * B], trq[:])
            trk = ps_tr.tile([128, B], BF16, tag="ps_tr", name="trk")
            nc.tensor.transpose(trk[:], kro[:], idb[0:B, 0:B])
            nc.vector.tensor_copy(knT[:], trk[:])
            for i in range(4):
                emit_splice(ORDER[i])

        # ---- phase 2: per-request attention ------------------------------
        with ExitStack() as s3:
            ps_qk = s3.enter_context(
                tc.tile_pool(name="ps_qk", bufs=3, space="PSUM"))
            ps_pv = s3.enter_context(
                tc.tile_pool(name="ps_pv", bufs=2, space="PSUM"))

            def emit_qk_chunk(b, qk, rq, c):
                L = Ls[b]
                KTt, _ = state[b]
                Lvc = min(128, L - c * 128)
                nc.tensor.matmul(qk[0:Lvc, c * G:(c + 1) * G],
                                 KTt[:, c * 128:c * 128 + Lvc], rq,
                                 start=True, stop=True)

            def emit_pv_chunk(b, pv, c, nch2):
                L = Ls[b]
                _, Vt = state[b]
                Lvc = min(128, L - c * 128)
                nc.tensor.matmul(pv[:],
                                 sc_all[0:Lvc, c * B * G + b * G:
                                        c * B * G + (b + 1) * G],
                                 Vt[0:Lvc, c * 128:(c + 1) * 128],
                                 start=(c == 0), stop=(c == nch2 - 1))

            def emit_qk_prologue(b):
                L = Ls[b]
                nch2 = (L + 127) // 128
                Lv = L - (nch2 - 1) * 128
                qk = ps_qk.tile([128, NSUB * G], F32, tag="ps_qk",
                                name=f"qk{b}")
                rq = qT[:].rearrange("p (g b) -> p g b", b=B)[:, :, b]
                if Lv < 128:
                    # pre-fill the tail chunk's block; the matmul then
                    # overwrites rows [0:Lv), leaving -inf beyond the
                    # context (PSUM accesses must be 32-partition-aligned,
                    # so a [Lv:128) memset is not expressible)
                    nc.vector.memset(qk[:, (nch2 - 1) * G:nch2 * G], -1e30)
                return qk, rq, nch2

            def emit_exp(b, qk, nch2):
                scv = sc_all[:].rearrange("p (c w) -> p c w", w=B * G)
                if nch2 < NSUB:
                    nc.vector.memset(
                        scv[:, nch2:NSUB, b * G:(b + 1) * G], 0.0)
                nc.scalar.activation(scv[:, 0:nch2, b * G:(b + 1) * G],
                                     qk[:, 0:nch2 * G],
                                     mybir.ActivationFunctionType.Exp,
                                     scale=SCALE)

            def emit_pv_epilogue(b, pv, last=False):
                state.pop(b)
                att = nrmpool.tile([G, HEAD_DIM], BF16, tag="att",
                                   name=f"att{b}")
                nc.vector.tensor_copy(att[:], pv[:])
                eng = nc.sync if last else nc.gpsimd
                eng.dma_start(pvn[G * b:G * (b + 1), :], att[:])

            # software pipeline: QK(b) chunk matmuls interleave with
            # PV(b-1) chunk matmuls so QK's 128-col LDWEIGHTS hides under
            # PV's 128-col moving stream (and vice versa)
            prev = None  # (b, pv_tile, nch)
            for i in range(B):
                b = ORDER[i]
                if i + 4 < B:
                    emit_load(ORDER[i + 4])
                if i in (6, 11, 16, 21):
                    emit_wo_load((i - 6) // 5)
                qk, rq, nch2 = emit_qk_prologue(b)
                if prev is None:
                    for c in range(nch2):
                        emit_qk_chunk(b, qk, rq, c)
                else:
                    pb, pv, pnch = prev
                    for c in range(max(nch2, pnch)):
                        if c < nch2:
                            emit_qk_chunk(b, qk, rq, c)
                        if c < pnch:
                            emit_pv_chunk(pb, pv, c, pnch)
                    emit_pv_epilogue(pb, pv)
                emit_exp(b, qk, nch2)
                pv = ps_pv.tile([G, HEAD_DIM], F32, tag="ps_pv",
                                name=f"pv{b}")
                prev = (b, pv, nch2)
            pb, pv, pnch = prev
            for c in range(pnch):
                emit_pv_chunk(pb, pv, c, pnch)
            emit_pv_epilogue(pb, pv, last=True)

        # ---- phase 3: batched softmax denominators + normalize -----------
        with ExitStack() as s4:
            npool = s4.enter_context(tc.tile_pool(name="norm", bufs=1))
            ps_den = s4.enter_context(
                tc.tile_pool(name="ps_den", bufs=1, space="PSUM"))
            ps_t = s4.enter_context(
                tc.tile_pool(name="ps_t", bufs=1, space="PSUM"))
            den_ps = ps_den.tile([128, 1], F32, tag="ps_den")
            for c in range(NSUB):
                nc.tensor.matmul(den_ps[:],
                                 sc_all[:, c * B * G:(c + 1) * B * G],
                                 onesD[:],
                                 start=(c == 0), stop=(c == NSUB - 1))
            rcp = npool.tile([128, 1], F32, tag="rcp")
            nc.vector.reciprocal(rcp[:], den_ps[:])
            pvm = npool.tile([128, 128], BF16, tag="pvm")
            nc.vector.tensor_scalar_mul(pvm[:], pvn[:], rcp[:])
            pvt_ps = ps_t.tile([128, 128], BF16, tag="ps_t")
            nc.tensor.transpose(pvt_ps[:], pvm[:], idb[:])
            nc.vector.tensor_copy(pvT[:], pvt_ps[:])

        # ---- phase 4: output projection ----------------------------------
        with ExitStack() as s5:
            outp = s5.enter_context(tc.tile_pool(name="outp", bufs=1))
            ps_o = s5.enter_context(
                tc.tile_pool(name="ps_o", bufs=8, space="PSUM"))
            out_sb = outp.tile([B, D], F32, tag="out")
            o_ps = [ps_o.tile([B, 512], F32, tag="ps_o", name=f"ops{n}")
                    for n in range(8)]
            pvr = pvT[:].rearrange("p (b g) -> p b g", g=G)
            for g in range(G):
                lt = pvr[:, :, g]
                for n in range(8):
                    nc.tensor.matmul(o_ps[n][:], lt,
                                     wo_tiles[g][:, n * 512:(n + 1) * 512],
                                     start=(g == 0), stop=(g == G - 1))
            for n in range(8):
                if n % 2 == 0:
                    nc.vector.tensor_copy(out_sb[:, n * 512:(n + 1) * 512],
                                          o_ps[n][:])
                else:
                    nc.scalar.copy(out_sb[:, n * 512:(n + 1) * 512],
                                   o_ps[n][:])
            nc.sync.dma_start(out_d[:], out_sb[:])

    nc.compile()
    return nc


def _prep_inputs(x, Wq, Wk, Wv, Wo, key_cache, value_cache, block_tables,
                 context_lens):
    import ml_dtypes
    bf16 = ml_dtypes.bfloat16
    x = np.asarray(x, dtype=np.float32).reshape(B, D)
    # xT[p, kc*32+b] = x[b, kc*128+p]
    xT = np.ascontiguousarray(
        x.reshape(B, 32, 128).transpose(2, 1, 0).reshape(128, 32 * B)
    ).astype(bf16)
    Wq = np.asarray(Wq, dtype=np.float32)
    Wk = np.asarray(Wk, dtype=np.float32)
    Wv = np.asarray(Wv, dtype=np.float32)
    Wo = np.asarray(Wo, dtype=np.float32)
    key_cache = np.asarray(key_cache, dtype=np.float32)
    value_cache = np.asarray(value_cache, dtype=np.float32)
    bt = np.asarray(block_tables, dtype=np.int64)
    cl = np.asarray(context_lens, dtype=np.int64)

    Ls = [int(v) for v in cl]
    pos = np.array([v - 1 for v in Ls], dtype=np.int64)

    # rope tables at the new token's position ([b, half] layout)
    half = HEAD_DIM // 2
    inv_freq = (1.0 / (ROPE_BASE ** (np.arange(half, dtype=np.float32) / half))
                ).astype(np.float32)
    ang = pos.astype(np.float32)[:, None] * inv_freq[None, :]
    cqb = np.ascontiguousarray(
        np.tile(np.cos(ang).astype(np.float32), (1, G)))  # [B, 4*64]
    sqb = np.ascontiguousarray(
        np.tile(np.sin(ang).astype(np.float32), (1, G)))
    identb = np.eye(128, dtype=np.float32).astype(bf16)

    # gather runs over t in [0, L-1) - the new token is spliced on device
    runs = [_kv_blocks(bt[b], Ls[b] - 1) for b in range(B)]

    in_maps = []
    for h in range(N_CORES):
        # relaid cache: row p of group g = K^T col d=p / V row t=p
        K = key_cache[:, h, :].reshape(NGRP, 128, HEAD_DIM)
        V = value_cache[:, h, :].reshape(NGRP, 128, HEAD_DIM)
        kt = np.ascontiguousarray(
            K.transpose(2, 0, 1).reshape(128, NGRP * 128)).astype(bf16)
        vv = np.ascontiguousarray(
            V.transpose(1, 0, 2).reshape(128, NGRP * 128)).astype(bf16)
        # fused [Wq | Wk | Wv] moving operand, per 128-row input chunk
        wqkv = np.empty((128, 32 * WQKV), dtype=np.float32)
        for kc in range(32):
            r0 = kc * 128
            wqkv[:, kc * WQKV:kc * WQKV + GD] = \
                Wq[r0:r0 + 128, h * GD:(h + 1) * GD]
            wqkv[:, kc * WQKV + GD:kc * WQKV + GD + HEAD_DIM] = \
                Wk[r0:r0 + 128, h * HEAD_DIM:(h + 1) * HEAD_DIM]
            wqkv[:, kc * WQKV + GD + HEAD_DIM:(kc + 1) * WQKV] = \
                Wv[r0:r0 + 128, h * HEAD_DIM:(h + 1) * HEAD_DIM]
        in_maps.append({
            "xT": xT,
            "wqkv": wqkv.astype(bf16),
            "wo": np.ascontiguousarray(Wo[h * GD:(h + 1) * GD, :]
                                       ).astype(bf16),
            "kt": kt, "vv": vv,
            "cqb": cqb, "sqb": sqb, "identb": identb,
        })
    return Ls, runs, in_maps


def kernel(x, Wq, Wk, Wv, Wo, key_cache, value_cache, block_tables,
           context_lens):
    global LAST_RESULTS
    Ls, runs, in_maps = _prep_inputs(
        x, Wq, Wk, Wv, Wo, key_cache, value_cache, block_tables,
        context_lens)
    nc = _build_nc(Ls, runs)
    res = run_bass_kernel_spmd(nc, in_maps, list(range(N_CORES)))
    LAST_RESULTS = res

    out = np.zeros((B, D), dtype=np.float32)
    for h in range(N_CORES):
        out += res.results[h]["out"]
    return np.ascontiguousarray(out.reshape(B, 1, D))
